# revision 33
# baseline (speedup 1.0000x reference)
"""LyraGemma3 sliding-window attention — Trainium2 Bass kernel, 8 NeuronCores.

Sharding: core = b*4 + h  (b in {0,1} batch, h in {0..3} head-group).
Each core owns vanilla head h, lyra head 4+h, kv head h for batch b and
produces output rows [512h, 512h+512) of batch b — disjoint slices, no
collectives.

v4 design:
- All matmul operands bf16; q/k/v and outC SBUF-resident (no DRAM spill).
- Persistent intermediates split per 512-token group so phase C's first
  q-tiles depend only on early phase-A groups (no whole-tile false deps).
- Masks applied multiplicatively after exp via GpSimd affine_select, and
  masked tiles compute only their valid query subrange (causal tiles
  shrink, window-edge tiles grow), with the T loop ordered so a
  full-range tile carries the PSUM-zeroing start flag.
- 1/x and 1/sqrt(x) computed as exp(-ln(x)) / exp(-0.5 ln(x)) on the
  Scalar engine: every activation (Square/Copy/Ln/Exp) lives in one ACT
  table, and the slow DVE reciprocal disappears from all critical paths.
- Phase A/C tails (rstd broadcast + rope, softmax normalize) are
  deferred into the next tile's instruction stream so PE never waits.
- wo streams into SBUF during phase C (address space reused from the
  phase-A weights); phase D runs m-block-major with deferred PSUM->SBUF
  copies so only the last output block's copy+DMA is exposed.
"""

import sys

sys.path.insert(0, "/opt/trn_rl_repo")

import numpy as np
import ml_dtypes

import concourse.bass as bass
import concourse.tile as tile
from concourse import mybir
from concourse.tile import ScopedClock

F32 = mybir.dt.float32
BF16 = mybir.dt.bfloat16
AF = mybir.ActivationFunctionType
ALU = mybir.AluOpType
NPBF = ml_dtypes.bfloat16

B, S, HID = 2, 2048, 2560
H, KV, D = 8, 4, 256
WINDOW = 1024
THETA = 10000.0
EPS = 1e-6
SCALING = 256.0 ** (-0.5)  # 1/16

NKC = HID // 128  # 20 contraction chunks for projections
NST = 8           # phase-A s-tiles of 256 tokens
NT = S // 128     # 16 key tiles of 128
NQ = 4            # attention q-tiles of 512


class SplitWaitTC(tile.TileContext):
    """This container's walrus encodes at most ONE semaphore wait per
    instruction; Tile emits multi-wait sync_info. Hoist extra waits onto
    preceding same-engine NOPs."""

    def _drain_and_barrier(self, tick_clock, wait_clock):
        nc = self.nc
        drain_inst = nc.sync.drain()
        wait_clock.add_sem_waits(
            drain_inst.ins, ScopedClock({None: tick_clock.global_clock})
        )
        self._split_multi_waits()
        nc.all_engine_barrier()
        popped = nc._tile_sem_poison_stack.pop()
        assert popped is self._sem_poison
        nc.clear_and_free_semaphores(list(self.sems.allocated().values()))
        nc.all_engine_barrier()

    def _split_multi_waits(self):
        nc = self.nc
        cur_bb = nc.cur_bb
        assert cur_bb is not None
        for f in nc.m.functions:
            for blk in f.blocks:
                insts = blk.instructions
                i = 0
                while i < len(insts):
                    inst = insts[i]
                    si = inst.sync_info
                    if si is not None and si.on_wait and len(si.on_wait) > 1:
                        waits = list(si.on_wait)
                        inst.sync_info = mybir.SyncInfo(
                            on_wait=waits[-1:], on_update=si.on_update
                        )
                        eng = inst.engine
                        for w in waits[:-1]:
                            nop = nc.engines[eng].nop()
                            nop.ins.sync_info = mybir.SyncInfo(
                                on_wait=[w], on_update=[]
                            )
                            cur_bb.bb.instructions.remove(nop.ins)
                            insts.insert(i, nop.ins)
                            i += 1
                    i += 1


def _c_tiles(Q):
    """Key-tile schedule for q-tile Q (queries [512Q, 512Q+512)).
    Returns [(T, off, length, select)] where [off, off+length) is the
    valid query subrange and select is None or (pattern, base, chan_mult)
    for the post-exp GpSimd affine_select. Ordered so the first entry is
    full-range (its matmul carries start=True and zeroes the whole PSUM
    region)."""
    out = []
    for T in range(max(0, 4 * Q - 4), 4 * Q):  # fully-valid tiles
        out.append((T, 0, 512, None))
    for j in range(4):  # causal diagonal: queries f >= 128 j are live
        ln = 512 - 128 * j
        # keep where f' - p >= 0 (f' is the index within the subrange)
        out.append((4 * Q + j, 128 * j, ln, ([[1, ln]], 0, -1)))
    if Q >= 2:
        for jp in range(4):  # window edge: queries f <= 128 jp + 126 live
            ln = 128 * jp + 128
            # keep where p - f + (128 jp - 1) >= 0
            out.append((4 * Q - 8 + jp, 0, ln, ([[-1, ln]], 128 * jp - 1, 1)))
    return out


def build_program():
    nc = bass.Bass()

    hsp = nc.declare_dram_parameter("hsp", [128, NST * NKC * 256], BF16, isOutput=False)
    wqp = nc.declare_dram_parameter("wqp", [4, 128, NKC * 128], BF16, isOutput=False)
    wkp = nc.declare_dram_parameter("wkp", [2, 128, NKC * 128], BF16, isOutput=False)
    wvp = nc.declare_dram_parameter("wvp", [128, NKC * 256], BF16, isOutput=False)
    wop = nc.declare_dram_parameter("wop", [128, 16 * HID], BF16, isOutput=False)
    cosp = nc.declare_dram_parameter("cosp", [128, S], BF16, isOutput=False)
    sinp = nc.declare_dram_parameter("sinp", [128, S], BF16, isOutput=False)
    invq_d = nc.declare_dram_parameter("invq", [128, 2], BF16, isOutput=False)
    invk_d = nc.declare_dram_parameter("invk", [128, 2], BF16, isOutput=False)
    onec_d = nc.declare_dram_parameter("onec", [128, 1], BF16, isOutput=False)
    oner_d = nc.declare_dram_parameter("oner", [1, 128], BF16, isOutput=False)
    epsb_d = nc.declare_dram_parameter("epsb", [1, 1], F32, isOutput=False)
    out_d = nc.declare_dram_parameter("out", [512, HID], F32, isOutput=True)

    with SplitWaitTC(nc) as tc:
        with tc.tile_pool(name="outer", bufs=1) as pO:
            # persistent intermediates, split per 512-token group g:
            # qTs[g]: [128, c*512 + t], c in {van d0, van d1, lyra d0, lyra d1}
            # kTrs/kTns[g]: [128, c*512 + t], c = d half
            # v_s[g]: [128, tloc*256 + d], tloc = key-tile within group
            qTs = [pO.tile([128, 4 * 512], BF16, name=f"qT{g}") for g in range(4)]
            kTrs = [pO.tile([128, 2 * 512], BF16, name=f"kTr{g}") for g in range(4)]
            kTns = [pO.tile([128, 2 * 512], BF16, name=f"kTn{g}") for g in range(4)]
            v_s = [pO.tile([128, 4 * 256], BF16, name=f"v{g}") for g in range(4)]
            outC = [
                [pO.tile([128, S], BF16, name=f"outC{s}{c}") for c in range(2)]
                for s in range(2)
            ]

            zero_fill = nc.gpsimd.to_reg(0.0)
            # Warm up the GpSimd engine: the first affine_select pays a
            # ~8us ucode-load cost; issue a dummy one before phase A so
            # phase C's first masked tile doesn't eat it.
            warm = pO.tile([128, 8], BF16, name="warm")
            nc.gpsimd.memset(warm[:], 0.0)
            nc.gpsimd.affine_select(
                warm[:], warm[:],
                pattern=[[1, 8]],
                compare_op=ALU.is_ge,
                fill=zero_fill,
                base=0,
                channel_multiplier=-1,
            )

            # Phase C/D SBUF temps live in a pool opened BEFORE phase A so
            # their addresses never overlap phase-A tiles: C's first ops
            # don't wait for the phase-A tail to release its space.
            pCt = tc.tile_pool(name="pCt", bufs=1)
            pC = pCt.__enter__()

            # ================= PHASE A: projections + norm + rope ========
            # 512-token s-tiles (4 of them): half the matmul / copy / rope
            # instruction count of a 256-token pipeline at the same FLOPs.
            with (
                tc.tile_pool(name="pA", bufs=1) as pA,
                tc.tile_pool(name="pAps", bufs=1, space="PSUM") as psA,
            ):
                # hidden states arrive as four quarter tiles per s-tile:
                # the first matmuls start after ~1.3MB of DMA, and the
                # quarters double-buffer across s-tiles.
                hstq_t = {}

                def hst_dma(g):
                    qs = [
                        pA.tile([128, 5 * 512], BF16, name=f"hstq{i}", bufs=2)
                        for i in range(4)
                    ]
                    for i in range(4):
                        nc.sync.dma_start(
                            qs[i][:],
                            hsp[:, (g * 4 + i) * 5 * 512 : (g * 4 + i + 1) * 5 * 512],
                        )
                    hstq_t[g] = qs

                wq_sb = [pA.tile([128, NKC * 128], BF16, name=f"wq{hc}") for hc in range(4)]
                qs0 = [
                    pA.tile([128, 5 * 512], BF16, name=f"hstq{i}", bufs=2)
                    for i in range(4)
                ]
                hstq_t[0] = qs0
                nc.sync.dma_start(qs0[0][:], hsp[:, 0 : 5 * 512])
                nc.sync.dma_start(wq_sb[0][:, 0 : 10 * 128], wqp[0][:, 0 : 10 * 128])
                nc.sync.dma_start(wq_sb[1][:, 0 : 10 * 128], wqp[1][:, 0 : 10 * 128])
                nc.sync.dma_start(qs0[1][:], hsp[:, 5 * 512 : 10 * 512])
                nc.sync.dma_start(
                    wq_sb[0][:, 10 * 128 : NKC * 128], wqp[0][:, 10 * 128 : NKC * 128]
                )
                nc.sync.dma_start(
                    wq_sb[1][:, 10 * 128 : NKC * 128], wqp[1][:, 10 * 128 : NKC * 128]
                )
                for i in range(2, 4):
                    nc.sync.dma_start(qs0[i][:], hsp[:, i * 5 * 512 : (i + 1) * 5 * 512])
                onec = pO.tile([128, 1], BF16, name="onec")
                nc.sync.dma_start(onec[:], onec_d[:])
                oner = pO.tile([1, 128], BF16, name="oner")
                nc.sync.dma_start(oner[:], oner_d[:])
                epsb = pO.tile([1, 1], F32, name="epsb")
                nc.sync.dma_start(epsb[:], epsb_d[:])
                invq = pO.tile([128, 2], BF16, name="invq")
                nc.sync.dma_start(invq[:], invq_d[:])
                invk = pO.tile([128, 2], BF16, name="invk")
                nc.sync.dma_start(invk[:], invk_d[:])
                for hc in range(2, 4):
                    nc.sync.dma_start(wq_sb[hc][:], wqp[hc])
                wk_sb = [pA.tile([128, NKC * 128], BF16, name=f"wk{hc}") for hc in range(2)]
                for hc in range(2):
                    nc.sync.dma_start(wk_sb[hc][:], wkp[hc])
                wv_sb = pA.tile([128, NKC * 256], BF16, name="wv_sb")
                nc.sync.dma_start(wv_sb[:], wvp[:])
                hst_dma(1)
                cos_sb = pA.tile([128, S], BF16, name="cos_sb")
                nc.sync.dma_start(cos_sb[:], cosp[:])
                sin_sb = pA.tile([128, S], BF16, name="sin_sb")
                nc.sync.dma_start(sin_sb[:], sinp[:])

                def hs_ap(g, kc, lo, width):
                    t = hstq_t[g][kc // 5]
                    base = (kc % 5) * 512
                    return t[:, base + lo : base + lo + width]

                prev_tail = None
                for g in range(4):
                    if g + 2 < 4:
                        hst_dma(g + 2)
                    # ---- projections (accumulate over 20 HID chunks) ----
                    qz = pA.tile([128, 2048], BF16, name="qz", bufs=2)
                    if g == 0:
                        # quarter-paced warm-up: two concurrent accumulation
                        # groups walk the chunks in DMA-arrival order
                        for pair in ((0, 1), (2, 3)):
                            paccs = {
                                hc: psA.tile([128, 512], F32, name="pacc", bufs=2)
                                for hc in pair
                            }
                            for qtr in range(4):
                                for hc in pair:
                                    for kc in range(qtr * 5, qtr * 5 + 5):
                                        nc.tensor.matmul(
                                            paccs[hc][:],
                                            wq_sb[hc][:, kc * 128 : (kc + 1) * 128],
                                            hs_ap(g, kc, 0, 512),
                                            start=(kc == 0),
                                            stop=(kc == NKC - 1),
                                            skip_group_check=True,
                                        )
                            for hc in pair:
                                nc.scalar.copy(
                                    qz[:, hc * 512 : (hc + 1) * 512], paccs[hc][:]
                                )
                    else:
                        for hc in range(4):
                            pq = psA.tile([128, 512], F32, name="pacc", bufs=2)
                            for kc in range(NKC):
                                nc.tensor.matmul(
                                    pq[:],
                                    wq_sb[hc][:, kc * 128 : (kc + 1) * 128],
                                    hs_ap(g, kc, 0, 512),
                                    start=(kc == 0),
                                    stop=(kc == NKC - 1),
                                )
                            nc.scalar.copy(qz[:, hc * 512 : (hc + 1) * 512], pq[:])
                    sqq = pA.tile([128, 2048], BF16, name="sqq", bufs=1)
                    nc.scalar.activation(sqq[:], qz[:], AF.Square)
                    kz = pA.tile([128, 1024], BF16, name="kz", bufs=2)
                    for hc in range(2):
                        pk = psA.tile([128, 512], F32, name="pacc", bufs=2)
                        for kc in range(NKC):
                            nc.tensor.matmul(
                                pk[:],
                                wk_sb[hc][:, kc * 128 : (kc + 1) * 128],
                                hs_ap(g, kc, 0, 512),
                                start=(kc == 0),
                                stop=(kc == NKC - 1),
                            )
                        nc.scalar.copy(kz[:, hc * 512 : (hc + 1) * 512], pk[:])
                    sqk = pA.tile([128, 1024], BF16, name="sqk", bufs=1)
                    nc.scalar.activation(sqk[:], kz[:], AF.Square)

                    def emit_pn(head):
                        pn = psA.tile([1, 512], F32, name="pn", bufs=2)
                        for c in range(2):
                            if head < 2:
                                rhs = sqq[:, (head * 2 + c) * 512 : (head * 2 + c + 1) * 512]
                                lhsT = invq[:, c : c + 1]
                            else:
                                rhs = sqk[:, c * 512 : (c + 1) * 512]
                                lhsT = invk[:, c : c + 1]
                            nc.tensor.matmul(
                                pn[:], lhsT, rhs, start=(c == 0), stop=(c == 1)
                            )
                        lnm = pA.tile([1, 512], F32, name="lnm", bufs=2)
                        nc.scalar.activation(
                            lnm[:], pn[:], AF.Ln, bias=epsb[:], scale=1.0 / 256.0
                        )
                        rst = pA.tile([1, 512], BF16, name="rst", bufs=2)
                        nc.scalar.activation(rst[:], lnm[:], AF.Exp, scale=-0.5)
                        return rst

                    rsts = [emit_pn(0), emit_pn(1)]
                    for sm in range(4):
                        pv = psA.tile([128, 256], F32, name="pvacc", bufs=2)
                        for kc in range(NKC):
                            nc.tensor.matmul(
                                pv[:],
                                hs_ap(g, kc, sm * 128, 128),
                                wv_sb[:, kc * 256 : (kc + 1) * 256],
                                start=(kc == 0),
                                stop=(kc == NKC - 1),
                            )
                        nc.scalar.copy(
                            v_s[g][:, sm * 256 : sm * 256 + 256], pv[:]
                        )

                    rsts.append(emit_pn(2))

                    # tail (rstd broadcast + rope) for the PREVIOUS s-tile:
                    # its norm chain has finished, so the pbc matmuls never
                    # stall PE, and rope (DVE) runs under this tile's
                    # projections.
                    def make_tail(g, qz, kz, rsts):
                        s0 = g * 512

                        def tail():
                            bcs = []
                            for head in range(3):
                                pbc = psA.tile([128, 512], F32, name="pbc", bufs=2)
                                nc.tensor.matmul(
                                    pbc[:], oner[:], rsts[head][:], start=True, stop=True
                                )
                                bc = pA.tile([128, 512], BF16, name=f"bc{head}", bufs=1)
                                nc.vector.tensor_copy(bc[:], pbc[:])
                                bcs.append(bc)
                            cs = cos_sb[:, s0 : s0 + 512]
                            sn = sin_sb[:, s0 : s0 + 512]

                            def rope2(z0, z1, bc, d0, d1):
                                t0 = pA.tile([128, 512], BF16, name="t0", bufs=1)
                                nc.vector.tensor_mul(t0[:], z0, cs)
                                t1 = pA.tile([128, 512], BF16, name="t1", bufs=1)
                                nc.vector.tensor_mul(t1[:], z1, sn)
                                u0 = pA.tile([128, 512], BF16, name="u0", bufs=1)
                                nc.vector.tensor_sub(u0[:], t0[:], t1[:])
                                nc.vector.tensor_mul(d0, u0[:], bc[:])
                                t2 = pA.tile([128, 512], BF16, name="t2", bufs=1)
                                nc.vector.tensor_mul(t2[:], z1, cs)
                                t3 = pA.tile([128, 512], BF16, name="t3", bufs=1)
                                nc.vector.tensor_mul(t3[:], z0, sn)
                                u1 = pA.tile([128, 512], BF16, name="u1", bufs=1)
                                nc.vector.tensor_add(u1[:], t2[:], t3[:])
                                nc.vector.tensor_mul(d1, u1[:], bc[:])

                            for head in range(2):
                                rope2(
                                    qz[:, (head * 2) * 512 : (head * 2) * 512 + 512],
                                    qz[:, (head * 2 + 1) * 512 : (head * 2 + 1) * 512 + 512],
                                    bcs[head],
                                    qTs[g][:, (head * 2) * 512 : (head * 2) * 512 + 512],
                                    qTs[g][:, (head * 2 + 1) * 512 : (head * 2 + 1) * 512 + 512],
                                )
                            rope2(
                                kz[:, 0:512], kz[:, 512:1024], bcs[2],
                                kTrs[g][:, 0:512],
                                kTrs[g][:, 512:1024],
                            )
                            nc.vector.tensor_mul(
                                kTns[g][:, 0:512], kz[:, 0:512], bcs[2][:]
                            )
                            nc.vector.tensor_mul(
                                kTns[g][:, 512:1024], kz[:, 512:1024], bcs[2][:]
                            )

                        return tail

                    if prev_tail is not None:
                        prev_tail()
                    prev_tail = make_tail(g, qz, kz, rsts)
                prev_tail()

            # ================= PHASES C+D ================================
            # wo streams into the SBUF space the phase-A pool released.
            with tc.tile_pool(name="pWo", bufs=1) as pWo:
                wo_sb = pWo.tile([128, 16 * HID], BF16, name="wo_sb")
                for i in range(4):
                    nc.sync.dma_start(
                        wo_sb[:, i * 4 * HID : (i + 1) * 4 * HID],
                        wop[:, i * 4 * HID : (i + 1) * 4 * HID],
                    )

                # ================= PHASE C: attention ====================
                with tc.tile_pool(name="pCps", bufs=1, space="PSUM") as psC:
                    # Pre-create the short-lived psum names so they occupy
                    # the LOW banks: phase D's first psum tiles then reuse
                    # banks whose last readers finish early, not the final
                    # normalize chain's po banks.
                    for _ in range(2):
                        psC.tile([128, 512], F32, name="pss", bufs=2)
                    psC.tile([1, 512], F32, name="psm", bufs=1)
                    psC.tile([128, 512], F32, name="pbcC", bufs=1)
                    prev_norm = None
                    for stream in range(2):  # 0 = vanilla (roped k), 1 = lyra
                        kTg = kTrs if stream == 0 else kTns
                        # Q0 is all causal-masked tiles; run it last so the
                        # exp->select chain is warm and pipelined when it hits.
                        for Q in (1, 2, 3, 0):
                            tiles = _c_tiles(Q)
                            po0 = psC.tile([128, 512], F32, name="po0", bufs=2)
                            po1 = psC.tile([128, 512], F32, name="po1", bufs=2)
                            psm = psC.tile([1, 512], F32, name="psm", bufs=1)
                            probs_t = {}

                            def emit_scores(i):
                                T, off, ln, sel = tiles[i]
                                kT = kTg[T // 4]
                                tl = (T % 4) * 128
                                pss = psC.tile([128, 512], F32, name="pss", bufs=2)
                                for c in range(2):
                                    qb = (2 * stream + c) * 512 + off
                                    nc.tensor.matmul(
                                        pss[:, off : off + ln],
                                        kT[:, c * 512 + tl : c * 512 + tl + 128],
                                        qTs[Q][:, qb : qb + ln],
                                        start=(c == 0),
                                        stop=(c == 1),
                                    )
                                probs = pC.tile([128, 512], BF16, name="probs", bufs=5)
                                if sel is None:
                                    nc.scalar.activation(
                                        probs[:, off : off + ln],
                                        pss[:, off : off + ln],
                                        AF.Exp,
                                        scale=SCALING,
                                    )
                                else:
                                    pattern, base, cm = sel
                                    praw = pC.tile([128, 512], BF16, name="praw", bufs=2)
                                    nc.scalar.activation(
                                        praw[:, off : off + ln],
                                        pss[:, off : off + ln],
                                        AF.Exp,
                                        scale=SCALING,
                                    )
                                    nc.gpsimd.affine_select(
                                        probs[:, off : off + ln],
                                        praw[:, off : off + ln],
                                        pattern=pattern,
                                        compare_op=ALU.is_ge,
                                        fill=zero_fill,
                                        base=base,
                                        channel_multiplier=cm,
                                    )
                                probs_t[i] = probs

                            def emit_av(i):
                                T, off, ln, _ = tiles[i]
                                probs = probs_t.pop(i)
                                vt = v_s[T // 4]
                                tl = (T % 4) * 256
                                first = i == 0
                                last = i == len(tiles) - 1
                                nc.tensor.matmul(
                                    psm[:, off : off + ln],
                                    onec[:],
                                    probs[:, off : off + ln],
                                    start=first,
                                    stop=last,
                                    skip_group_check=True,
                                )
                                nc.tensor.matmul(
                                    po0[:, off : off + ln],
                                    vt[:, tl : tl + 128],
                                    probs[:, off : off + ln],
                                    start=first,
                                    stop=last,
                                    skip_group_check=True,
                                )
                                nc.tensor.matmul(
                                    po1[:, off : off + ln],
                                    vt[:, tl + 128 : tl + 256],
                                    probs[:, off : off + ln],
                                    start=first,
                                    stop=last,
                                    skip_group_check=True,
                                )

                            # software pipeline: AV for tile i trails the
                            # scores for tile i+2 so PE never waits on the
                            # ACT exp / GpSimd mask chain; the previous
                            # q-tile's normalize chain is emitted after the
                            # first scores block for the same reason.
                            for i in range(len(tiles)):
                                emit_scores(i)
                                if i == 0 and prev_norm is not None:
                                    prev_norm()
                                if i >= 3:
                                    emit_av(i - 3)
                            emit_av(len(tiles) - 3)
                            emit_av(len(tiles) - 2)
                            emit_av(len(tiles) - 1)
                            # free the single psm bank ASAP (ACT copy); the
                            # rest of the normalize chain is deferred into
                            # the next q-tile's score stream.
                            psmb = pC.tile([1, 512], BF16, name="psmb", bufs=2)
                            nc.scalar.copy(psmb[:], psm[:])

                            def make_norm(stream, Q, po0, po1, psmb):
                                def norm():
                                    pbcC = psC.tile(
                                        [128, 512], F32, name="pbcC", bufs=1
                                    )
                                    nc.tensor.matmul(
                                        pbcC[:], oner[:], psmb[:], start=True, stop=True
                                    )
                                    # 1/x as exp(-ln(x)), both on ACT
                                    lnC = pC.tile([128, 512], F32, name="lnC", bufs=2)
                                    nc.scalar.activation(lnC[:], pbcC[:], AF.Ln)
                                    bcsC = pC.tile([128, 512], F32, name="bcsC", bufs=2)
                                    nc.scalar.activation(
                                        bcsC[:], lnC[:], AF.Exp, scale=-1.0
                                    )
                                    for dc in range(2):
                                        po = po0 if dc == 0 else po1
                                        nc.vector.tensor_mul(
                                            outC[stream][dc][:, Q * 512 : (Q + 1) * 512],
                                            po[:],
                                            bcsC[:],
                                        )

                                return norm

                            prev_norm = make_norm(stream, Q, po0, po1, psmb)
                    prev_norm()

                # ================= PHASE D: output projection ============
                with tc.tile_pool(name="pDps", bufs=1, space="PSUM") as psD:
                    # outC is stored contiguously in query order; the lhsT
                    # for contraction chunk (j, dc) is the stride-8 view
                    # q = 8m + j over rows m0..m0+128. m-block-major with
                    # copies deferred one block so only the last block's
                    # copy+DMA is exposed.
                    prev_copy = None
                    for co in range(0, HID, 512):
                        for m in range(4):
                            stream, m0 = m // 2, (m % 2) * 128
                            pos = psD.tile([128, 512], F32, name=f"pD{m}", bufs=2)
                            for kc in range(16):
                                j, dc = kc // 2, kc % 2
                                lhsT = (
                                    outC[stream][dc][:]
                                    .rearrange("p (m j) -> p m j", j=8)
                                    [:, m0 : m0 + 128, j : j + 1]
                                )
                                nc.tensor.matmul(
                                    pos[:],
                                    lhsT,
                                    wo_sb[:, kc * HID + co : kc * HID + co + 512],
                                    start=(kc == 0),
                                    stop=(kc == 15),
                                )

                            def make_copy(co, m, pos):
                                last = co == HID - 512 and m == 3

                                def docopy():
                                    if last:
                                        for hh in range(2):
                                            osth = pC.tile(
                                                [128, 256], F32, name="osth", bufs=2
                                            )
                                            nc.scalar.copy(
                                                osth[:],
                                                pos[:, hh * 256 : (hh + 1) * 256],
                                            )
                                            nc.sync.dma_start(
                                                out_d[
                                                    m * 128 : (m + 1) * 128,
                                                    co + hh * 256 : co + (hh + 1) * 256,
                                                ],
                                                osth[:],
                                            )
                                    else:
                                        ost = pC.tile(
                                            [128, 512], F32, name="ost", bufs=5
                                        )
                                        nc.scalar.copy(ost[:], pos[:])
                                        nc.sync.dma_start(
                                            out_d[
                                                m * 128 : (m + 1) * 128, co : co + 512
                                            ],
                                            ost[:],
                                        )

                                return docopy

                            if prev_copy is not None:
                                prev_copy()
                            prev_copy = make_copy(co, m, pos)
                    prev_copy()
            pCt.__exit__(None, None, None)
    return nc


def _host_inputs(hidden_states, wq, wk, wv, wo, q_norm_w, k_norm_w):
    """Build the 8 per-core input maps (all host-side numpy prep).
    Every tensor is prepacked into its exact SBUF layout so device DMAs
    are plain contiguous copies."""
    hs = np.asarray(hidden_states, dtype=np.float32)
    wq = np.asarray(wq, dtype=np.float32)
    wk = np.asarray(wk, dtype=np.float32)
    wv = np.asarray(wv, dtype=np.float32)
    wo = np.asarray(wo, dtype=np.float32)
    qnw = np.asarray(q_norm_w, dtype=np.float32)
    knw = np.asarray(k_norm_w, dtype=np.float32)

    def pack_w(w):  # [HID, width] -> [128, NKC*width] chunk-major free axis
        width = w.shape[1]
        return np.ascontiguousarray(
            w.reshape(NKC, 128, width).transpose(1, 0, 2).reshape(128, NKC * width)
        ).astype(NPBF)

    # hsT packed per s-tile: [128, (st, kc, 256)]
    hsp = []
    for b in range(B):
        h = hs[b].T.reshape(NKC, 128, 4, 512).transpose(1, 2, 0, 3)
        hsp.append(np.ascontiguousarray(h.reshape(128, 4 * NKC * 512)).astype(NPBF))

    inv_freq = 1.0 / (THETA ** (np.arange(0, D, 2, dtype=np.float32) / D))
    ang = np.outer(inv_freq, np.arange(S, dtype=np.float32))  # (128, S)
    cosp = np.ascontiguousarray(np.cos(ang)).astype(NPBF)
    sinp = np.ascontiguousarray(np.sin(ang)).astype(NPBF)

    invq = np.ascontiguousarray(((1.0 + qnw) ** -2).reshape(2, 128).T).astype(NPBF)
    invk = np.ascontiguousarray(((1.0 + knw) ** -2).reshape(2, 128).T).astype(NPBF)
    onec = np.ones((128, 1), NPBF)
    oner = np.ones((1, 128), NPBF)
    epsb = np.full((1, 1), EPS, np.float32)

    # wo packed: [128, (kc, HID)]
    wop = np.ascontiguousarray(
        wo.reshape(16, 128, HID).transpose(1, 0, 2).reshape(128, 16 * HID)
    ).astype(NPBF)

    qs = 1.0 + qnw
    ks = 1.0 + knw
    in_maps = []
    for core in range(8):
        b, h = core // 4, core % 4
        wq2 = np.concatenate(
            [
                wq[:, h * D : (h + 1) * D] * qs[None, :],
                wq[:, (4 + h) * D : (5 + h) * D] * qs[None, :],
            ],
            axis=1,
        )  # [HID, 512]
        wqp_ = np.stack([pack_w(wq2[:, hc * 128 : (hc + 1) * 128]) for hc in range(4)])
        wk1 = wk[:, h * D : (h + 1) * D] * ks[None, :]
        wkp_ = np.stack([pack_w(wk1[:, hc * 128 : (hc + 1) * 128]) for hc in range(2)])
        wvp_ = pack_w(wv[:, h * D : (h + 1) * D])
        in_maps.append(
            {
                "hsp": hsp[b],
                "wqp": wqp_,
                "wkp": wkp_,
                "wvp": wvp_,
                "wop": wop,
                "cosp": cosp,
                "sinp": sinp,
                "invq": invq,
                "invk": invk,
                "onec": onec,
                "oner": oner,
                "epsb": epsb,
            }
        )
    return in_maps


_PROGRAM = None


def kernel(hidden_states, wq, wk, wv, wo, q_norm_w, k_norm_w):
    global _PROGRAM
    from concourse.bass_utils import run_bass_kernel_spmd

    if _PROGRAM is None:
        _PROGRAM = build_program()
    in_maps = _host_inputs(hidden_states, wq, wk, wv, wo, q_norm_w, k_norm_w)
    res = run_bass_kernel_spmd(_PROGRAM, in_maps, core_ids=list(range(8)))
    out = np.empty((B, S, HID), np.float32)
    for core in range(8):
        b, h = core // 4, core % 4
        out[b, h * 512 : (h + 1) * 512, :] = res.results[core]["out"]
    return out


# revision 34
# speedup vs baseline: 1.1710x; 1.1710x over previous
"""LyraGemma3 sliding-window attention — Trainium2 Bass kernel, 8 NeuronCores.

Sharding: core = b*4 + h  (b in {0,1} batch, h in {0..3} head-group).
Each core owns vanilla head h, lyra head 4+h, kv head h for batch b and
produces output rows [512h, 512h+512) of batch b — disjoint slices, no
collectives.

v4 design:
- All matmul operands bf16; q/k/v and outC SBUF-resident (no DRAM spill).
- Persistent intermediates split per 512-token group so phase C's first
  q-tiles depend only on early phase-A groups (no whole-tile false deps).
- Masks applied multiplicatively after exp via GpSimd affine_select, and
  masked tiles compute only their valid query subrange (causal tiles
  shrink, window-edge tiles grow), with the T loop ordered so a
  full-range tile carries the PSUM-zeroing start flag.
- 1/x and 1/sqrt(x) computed as exp(-ln(x)) / exp(-0.5 ln(x)) on the
  Scalar engine: every activation (Square/Copy/Ln/Exp) lives in one ACT
  table, and the slow DVE reciprocal disappears from all critical paths.
- Phase A/C tails (rstd broadcast + rope, softmax normalize) are
  deferred into the next tile's instruction stream so PE never waits.
- wo streams into SBUF during phase C (address space reused from the
  phase-A weights); phase D runs m-block-major with deferred PSUM->SBUF
  copies so only the last output block's copy+DMA is exposed.
"""

import sys

sys.path.insert(0, "/opt/trn_rl_repo")

import numpy as np
import ml_dtypes

import concourse.bass as bass
import concourse.tile as tile
from concourse import mybir
from concourse.tile import ScopedClock

F32 = mybir.dt.float32
BF16 = mybir.dt.bfloat16
AF = mybir.ActivationFunctionType
ALU = mybir.AluOpType
NPBF = ml_dtypes.bfloat16

B, S, HID = 2, 2048, 2560
H, KV, D = 8, 4, 256
WINDOW = 1024
THETA = 10000.0
EPS = 1e-6
SCALING = 256.0 ** (-0.5)  # 1/16

NKC = HID // 128  # 20 contraction chunks for projections
NST = 8           # phase-A s-tiles of 256 tokens
NT = S // 128     # 16 key tiles of 128
NQ = 4            # attention q-tiles of 512


class SplitWaitTC(tile.TileContext):
    """This container's walrus encodes at most ONE semaphore wait per
    instruction; Tile emits multi-wait sync_info. Hoist extra waits onto
    preceding same-engine NOPs."""

    def _drain_and_barrier(self, tick_clock, wait_clock):
        nc = self.nc
        drain_inst = nc.sync.drain()
        wait_clock.add_sem_waits(
            drain_inst.ins, ScopedClock({None: tick_clock.global_clock})
        )
        self._split_multi_waits()
        nc.all_engine_barrier()
        popped = nc._tile_sem_poison_stack.pop()
        assert popped is self._sem_poison
        nc.clear_and_free_semaphores(list(self.sems.allocated().values()))
        nc.all_engine_barrier()

    def _split_multi_waits(self):
        nc = self.nc
        cur_bb = nc.cur_bb
        assert cur_bb is not None
        for f in nc.m.functions:
            for blk in f.blocks:
                insts = blk.instructions
                i = 0
                while i < len(insts):
                    inst = insts[i]
                    si = inst.sync_info
                    if si is not None and si.on_wait and len(si.on_wait) > 1:
                        waits = list(si.on_wait)
                        inst.sync_info = mybir.SyncInfo(
                            on_wait=waits[-1:], on_update=si.on_update
                        )
                        eng = inst.engine
                        for w in waits[:-1]:
                            nop = nc.engines[eng].nop()
                            nop.ins.sync_info = mybir.SyncInfo(
                                on_wait=[w], on_update=[]
                            )
                            cur_bb.bb.instructions.remove(nop.ins)
                            insts.insert(i, nop.ins)
                            i += 1
                    i += 1


def _c_tiles(Q):
    """Key-tile schedule for q-tile Q (queries [512Q, 512Q+512)).
    Returns [(T, off, length, select)] where [off, off+length) is the
    valid query subrange and select is None or (pattern, base, chan_mult)
    for the post-exp GpSimd affine_select. Ordered so the first entry is
    full-range (its matmul carries start=True and zeroes the whole PSUM
    region)."""
    out = []
    for T in range(max(0, 4 * Q - 4), 4 * Q):  # fully-valid tiles
        out.append((T, 0, 512, None))
    for j in range(4):  # causal diagonal: queries f >= 128 j are live
        ln = 512 - 128 * j
        # keep where f' - p >= 0 (f' is the index within the subrange)
        out.append((4 * Q + j, 128 * j, ln, ([[1, ln]], 0, -1)))
    if Q >= 2:
        for jp in range(4):  # window edge: queries f <= 128 jp + 126 live
            ln = 128 * jp + 128
            # keep where p - f + (128 jp - 1) >= 0
            out.append((4 * Q - 8 + jp, 0, ln, ([[-1, ln]], 128 * jp - 1, 1)))
    return out


def build_program():
    nc = bass.Bass()

    hsp = nc.declare_dram_parameter("hsp", [128, NST * NKC * 256], BF16, isOutput=False)
    wqp = nc.declare_dram_parameter("wqp", [4, 128, NKC * 128], BF16, isOutput=False)
    wkp = nc.declare_dram_parameter("wkp", [2, 128, NKC * 128], BF16, isOutput=False)
    wvp = nc.declare_dram_parameter("wvp", [128, NKC * 256], BF16, isOutput=False)
    wop = nc.declare_dram_parameter("wop", [128, 16 * HID], BF16, isOutput=False)
    cosp = nc.declare_dram_parameter("cosp", [128, S], BF16, isOutput=False)
    sinp = nc.declare_dram_parameter("sinp", [128, S], BF16, isOutput=False)
    invq_d = nc.declare_dram_parameter("invq", [128, 2], BF16, isOutput=False)
    invk_d = nc.declare_dram_parameter("invk", [128, 2], BF16, isOutput=False)
    onec_d = nc.declare_dram_parameter("onec", [128, 1], BF16, isOutput=False)
    oner_d = nc.declare_dram_parameter("oner", [1, 128], BF16, isOutput=False)
    epsb_d = nc.declare_dram_parameter("epsb", [1, 1], F32, isOutput=False)
    out_d = nc.declare_dram_parameter("out", [512, HID], F32, isOutput=True)

    with SplitWaitTC(nc) as tc:
        with tc.tile_pool(name="outer", bufs=1) as pO:
            # persistent intermediates, split per 512-token group g:
            # qTs[g]: [128, c*512 + t], c in {van d0, van d1, lyra d0, lyra d1}
            # kTrs/kTns[g]: [128, c*512 + t], c = d half
            # v_s[g]: [128, tloc*256 + d], tloc = key-tile within group
            qTs = [pO.tile([128, 4 * 512], BF16, name=f"qT{g}") for g in range(4)]
            kTrs = [pO.tile([128, 2 * 512], BF16, name=f"kTr{g}") for g in range(4)]
            kTns = [pO.tile([128, 2 * 512], BF16, name=f"kTn{g}") for g in range(4)]
            v_s = [pO.tile([128, 4 * 256], BF16, name=f"v{g}") for g in range(4)]
            outC = [
                [pO.tile([128, S], BF16, name=f"outC{s}{c}") for c in range(2)]
                for s in range(2)
            ]

            zero_fill = nc.gpsimd.to_reg(0.0)
            # Warm up the GpSimd engine: the first affine_select pays a
            # ~8us ucode-load cost; issue a dummy one before phase A so
            # phase C's first masked tile doesn't eat it.
            warm = pO.tile([128, 8], BF16, name="warm")
            nc.gpsimd.memset(warm[:], 0.0)
            nc.gpsimd.affine_select(
                warm[:], warm[:],
                pattern=[[1, 8]],
                compare_op=ALU.is_ge,
                fill=zero_fill,
                base=0,
                channel_multiplier=-1,
            )

            # Phase C/D SBUF temps live in a pool opened BEFORE phase A so
            # their addresses never overlap phase-A tiles: C's first ops
            # don't wait for the phase-A tail to release its space.
            pCt = tc.tile_pool(name="pCt", bufs=1)
            pC = pCt.__enter__()

            # ================= PHASE A: projections + norm + rope ========
            # 512-token s-tiles (4 of them): half the matmul / copy / rope
            # instruction count of a 256-token pipeline at the same FLOPs.
            with (
                tc.tile_pool(name="pA", bufs=1) as pA,
                tc.tile_pool(name="pAps", bufs=1, space="PSUM") as psA,
            ):
                # hidden states arrive as four quarter tiles per s-tile:
                # the first matmuls start after ~1.3MB of DMA, and the
                # quarters double-buffer across s-tiles.
                hstq_t = {}

                def hst_dma(g):
                    qs = [
                        pA.tile([128, 5 * 512], BF16, name=f"hstq{i}", bufs=2)
                        for i in range(4)
                    ]
                    for i in range(4):
                        nc.sync.dma_start(
                            qs[i][:],
                            hsp[:, (g * 4 + i) * 5 * 512 : (g * 4 + i + 1) * 5 * 512],
                        )
                    hstq_t[g] = qs

                wq_sb = [pA.tile([128, NKC * 128], BF16, name=f"wq{hc}") for hc in range(4)]
                qs0 = [
                    pA.tile([128, 5 * 512], BF16, name=f"hstq{i}", bufs=2)
                    for i in range(4)
                ]
                hstq_t[0] = qs0
                nc.sync.dma_start(qs0[0][:], hsp[:, 0 : 5 * 512])
                nc.sync.dma_start(wq_sb[0][:, 0 : 10 * 128], wqp[0][:, 0 : 10 * 128])
                nc.sync.dma_start(wq_sb[1][:, 0 : 10 * 128], wqp[1][:, 0 : 10 * 128])
                nc.sync.dma_start(qs0[1][:], hsp[:, 5 * 512 : 10 * 512])
                nc.sync.dma_start(
                    wq_sb[0][:, 10 * 128 : NKC * 128], wqp[0][:, 10 * 128 : NKC * 128]
                )
                nc.sync.dma_start(
                    wq_sb[1][:, 10 * 128 : NKC * 128], wqp[1][:, 10 * 128 : NKC * 128]
                )
                for i in range(2, 4):
                    nc.sync.dma_start(qs0[i][:], hsp[:, i * 5 * 512 : (i + 1) * 5 * 512])
                onec = pO.tile([128, 1], BF16, name="onec")
                nc.sync.dma_start(onec[:], onec_d[:])
                oner = pO.tile([1, 128], BF16, name="oner")
                nc.sync.dma_start(oner[:], oner_d[:])
                epsb = pO.tile([1, 1], F32, name="epsb")
                nc.sync.dma_start(epsb[:], epsb_d[:])
                invq = pO.tile([128, 2], BF16, name="invq")
                nc.sync.dma_start(invq[:], invq_d[:])
                invk = pO.tile([128, 2], BF16, name="invk")
                nc.sync.dma_start(invk[:], invk_d[:])
                for hc in range(2, 4):
                    nc.sync.dma_start(wq_sb[hc][:], wqp[hc])
                wk_sb = [pA.tile([128, NKC * 128], BF16, name=f"wk{hc}") for hc in range(2)]
                for hc in range(2):
                    nc.sync.dma_start(wk_sb[hc][:], wkp[hc])
                wv_sb = pA.tile([128, NKC * 256], BF16, name="wv_sb")
                nc.sync.dma_start(wv_sb[:], wvp[:])
                hst_dma(1)
                cos_sb = pA.tile([128, S], BF16, name="cos_sb")
                nc.sync.dma_start(cos_sb[:], cosp[:])
                sin_sb = pA.tile([128, S], BF16, name="sin_sb")
                nc.sync.dma_start(sin_sb[:], sinp[:])

                def hs_ap(g, kc, lo, width):
                    t = hstq_t[g][kc // 5]
                    base = (kc % 5) * 512
                    return t[:, base + lo : base + lo + width]

                prev_tail = None
                for g in range(4):
                    if g + 2 < 4:
                        hst_dma(g + 2)
                    # ---- projections (accumulate over 20 HID chunks) ----
                    qz = pA.tile([128, 2048], BF16, name="qz", bufs=2)
                    if g == 0:
                        # quarter-paced warm-up: two concurrent accumulation
                        # groups walk the chunks in DMA-arrival order
                        for pair in ((0, 1), (2, 3)):
                            paccs = {
                                hc: psA.tile([128, 512], F32, name="pacc", bufs=2)
                                for hc in pair
                            }
                            for qtr in range(4):
                                for hc in pair:
                                    for kc in range(qtr * 5, qtr * 5 + 5):
                                        nc.tensor.matmul(
                                            paccs[hc][:],
                                            wq_sb[hc][:, kc * 128 : (kc + 1) * 128],
                                            hs_ap(g, kc, 0, 512),
                                            start=(kc == 0),
                                            stop=(kc == NKC - 1),
                                            skip_group_check=True,
                                        )
                            for hc in pair:
                                nc.scalar.copy(
                                    qz[:, hc * 512 : (hc + 1) * 512], paccs[hc][:]
                                )
                    else:
                        for hc in range(4):
                            pq = psA.tile([128, 512], F32, name="pacc", bufs=2)
                            for kc in range(NKC):
                                nc.tensor.matmul(
                                    pq[:],
                                    wq_sb[hc][:, kc * 128 : (kc + 1) * 128],
                                    hs_ap(g, kc, 0, 512),
                                    start=(kc == 0),
                                    stop=(kc == NKC - 1),
                                )
                            nc.scalar.copy(qz[:, hc * 512 : (hc + 1) * 512], pq[:])
                    sqq = pA.tile([128, 2048], BF16, name="sqq", bufs=1)
                    nc.scalar.activation(sqq[:], qz[:], AF.Square)
                    kz = pA.tile([128, 1024], BF16, name="kz", bufs=2)
                    for hc in range(2):
                        pk = psA.tile([128, 512], F32, name="pacc", bufs=2)
                        for kc in range(NKC):
                            nc.tensor.matmul(
                                pk[:],
                                wk_sb[hc][:, kc * 128 : (kc + 1) * 128],
                                hs_ap(g, kc, 0, 512),
                                start=(kc == 0),
                                stop=(kc == NKC - 1),
                            )
                        nc.scalar.copy(kz[:, hc * 512 : (hc + 1) * 512], pk[:])
                    sqk = pA.tile([128, 1024], BF16, name="sqk", bufs=1)
                    nc.scalar.activation(sqk[:], kz[:], AF.Square)

                    def emit_pn(head):
                        pn = psA.tile([1, 512], F32, name="pn", bufs=2)
                        for c in range(2):
                            if head < 2:
                                rhs = sqq[:, (head * 2 + c) * 512 : (head * 2 + c + 1) * 512]
                                lhsT = invq[:, c : c + 1]
                            else:
                                rhs = sqk[:, c * 512 : (c + 1) * 512]
                                lhsT = invk[:, c : c + 1]
                            nc.tensor.matmul(
                                pn[:], lhsT, rhs, start=(c == 0), stop=(c == 1)
                            )
                        lnm = pA.tile([1, 512], F32, name="lnm", bufs=2)
                        nc.scalar.activation(
                            lnm[:], pn[:], AF.Ln, bias=epsb[:], scale=1.0 / 256.0
                        )
                        rst = pA.tile([1, 512], BF16, name="rst", bufs=2)
                        nc.scalar.activation(rst[:], lnm[:], AF.Exp, scale=-0.5)
                        return rst

                    rsts = [emit_pn(0), emit_pn(1)]
                    for sm in range(4):
                        pv = psA.tile([128, 256], F32, name="pvacc", bufs=2)
                        for kc in range(NKC):
                            nc.tensor.matmul(
                                pv[:],
                                hs_ap(g, kc, sm * 128, 128),
                                wv_sb[:, kc * 256 : (kc + 1) * 256],
                                start=(kc == 0),
                                stop=(kc == NKC - 1),
                            )
                        nc.scalar.copy(
                            v_s[g][:, sm * 256 : sm * 256 + 256], pv[:]
                        )

                    rsts.append(emit_pn(2))

                    # tail (rstd broadcast + rope) for the PREVIOUS s-tile:
                    # its norm chain has finished, so the pbc matmuls never
                    # stall PE, and rope (DVE) runs under this tile's
                    # projections.
                    def make_tail(g, qz, kz, rsts):
                        s0 = g * 512

                        def tail():
                            bcs = []
                            for head in range(3):
                                pbc = psA.tile([128, 512], F32, name="pbc", bufs=2)
                                nc.tensor.matmul(
                                    pbc[:], oner[:], rsts[head][:], start=True, stop=True
                                )
                                bc = pA.tile([128, 512], BF16, name=f"bc{head}", bufs=1)
                                nc.vector.tensor_copy(bc[:], pbc[:])
                                bcs.append(bc)
                            cs = cos_sb[:, s0 : s0 + 512]
                            sn = sin_sb[:, s0 : s0 + 512]

                            def rope2(z0, z1, bc, d0, d1):
                                t0 = pA.tile([128, 512], BF16, name="t0", bufs=1)
                                nc.vector.tensor_mul(t0[:], z0, cs)
                                t1 = pA.tile([128, 512], BF16, name="t1", bufs=1)
                                nc.vector.tensor_mul(t1[:], z1, sn)
                                u0 = pA.tile([128, 512], BF16, name="u0", bufs=1)
                                nc.vector.tensor_sub(u0[:], t0[:], t1[:])
                                nc.vector.tensor_mul(d0, u0[:], bc[:])
                                t2 = pA.tile([128, 512], BF16, name="t2", bufs=1)
                                nc.vector.tensor_mul(t2[:], z1, cs)
                                t3 = pA.tile([128, 512], BF16, name="t3", bufs=1)
                                nc.vector.tensor_mul(t3[:], z0, sn)
                                u1 = pA.tile([128, 512], BF16, name="u1", bufs=1)
                                nc.vector.tensor_add(u1[:], t2[:], t3[:])
                                nc.vector.tensor_mul(d1, u1[:], bc[:])

                            for head in range(2):
                                rope2(
                                    qz[:, (head * 2) * 512 : (head * 2) * 512 + 512],
                                    qz[:, (head * 2 + 1) * 512 : (head * 2 + 1) * 512 + 512],
                                    bcs[head],
                                    qTs[g][:, (head * 2) * 512 : (head * 2) * 512 + 512],
                                    qTs[g][:, (head * 2 + 1) * 512 : (head * 2 + 1) * 512 + 512],
                                )
                            rope2(
                                kz[:, 0:512], kz[:, 512:1024], bcs[2],
                                kTrs[g][:, 0:512],
                                kTrs[g][:, 512:1024],
                            )
                            nc.vector.tensor_mul(
                                kTns[g][:, 0:512], kz[:, 0:512], bcs[2][:]
                            )
                            nc.vector.tensor_mul(
                                kTns[g][:, 512:1024], kz[:, 512:1024], bcs[2][:]
                            )

                        return tail

                    if prev_tail is not None:
                        prev_tail()
                    prev_tail = make_tail(g, qz, kz, rsts)
                prev_tail()

            # ================= PHASES C+D ================================
            # wo streams into the SBUF space the phase-A pool released.
            with tc.tile_pool(name="pWo", bufs=1) as pWo:
                wo_sb = pWo.tile([128, 16 * HID], BF16, name="wo_sb")
                for i in range(4):
                    nc.sync.dma_start(
                        wo_sb[:, i * 4 * HID : (i + 1) * 4 * HID],
                        wop[:, i * 4 * HID : (i + 1) * 4 * HID],
                    )

                # ================= PHASE C: attention ====================
                with tc.tile_pool(name="pCps", bufs=1, space="PSUM") as psC:
                    # Pre-create every psum name in an order that maps the
                    # first q-tile's tiles (pss A/B, psm, po0 A, po1 A) onto
                    # phase-A banks whose last readers finish mid-tile
                    # (pacc/pvacc/pn, read by ACT), while the banks held
                    # until the phase-A tail's DVE reads (pbc) are only
                    # reused from the second q-tile on (po0/po1 buffer B)
                    # or by pbcC (first used at the first normalize).
                    for _ in range(2):
                        psC.tile([128, 512], F32, name="pss", bufs=2)
                    psC.tile([1, 512], F32, name="psm", bufs=1)
                    psC.tile([128, 512], F32, name="po0", bufs=2)
                    psC.tile([128, 512], F32, name="po1", bufs=2)
                    psC.tile([128, 512], F32, name="po0", bufs=2)
                    psC.tile([128, 512], F32, name="po1", bufs=2)
                    psC.tile([128, 512], F32, name="pbcC", bufs=1)
                    prev_norm = None
                    for stream in range(2):  # 0 = vanilla (roped k), 1 = lyra
                        kTg = kTrs if stream == 0 else kTns
                        # Q0 is all causal-masked tiles; run it last so the
                        # exp->select chain is warm and pipelined when it hits.
                        for Q in (1, 2, 3, 0):
                            tiles = _c_tiles(Q)
                            po0 = psC.tile([128, 512], F32, name="po0", bufs=2)
                            po1 = psC.tile([128, 512], F32, name="po1", bufs=2)
                            psm = psC.tile([1, 512], F32, name="psm", bufs=1)
                            probs_t = {}

                            def emit_scores(i):
                                T, off, ln, sel = tiles[i]
                                kT = kTg[T // 4]
                                tl = (T % 4) * 128
                                pss = psC.tile([128, 512], F32, name="pss", bufs=2)
                                for c in range(2):
                                    qb = (2 * stream + c) * 512 + off
                                    nc.tensor.matmul(
                                        pss[:, off : off + ln],
                                        kT[:, c * 512 + tl : c * 512 + tl + 128],
                                        qTs[Q][:, qb : qb + ln],
                                        start=(c == 0),
                                        stop=(c == 1),
                                    )
                                probs = pC.tile([128, 512], BF16, name="probs", bufs=5)
                                if sel is None:
                                    nc.scalar.activation(
                                        probs[:, off : off + ln],
                                        pss[:, off : off + ln],
                                        AF.Exp,
                                        scale=SCALING,
                                    )
                                else:
                                    pattern, base, cm = sel
                                    praw = pC.tile([128, 512], BF16, name="praw", bufs=2)
                                    nc.scalar.activation(
                                        praw[:, off : off + ln],
                                        pss[:, off : off + ln],
                                        AF.Exp,
                                        scale=SCALING,
                                    )
                                    nc.gpsimd.affine_select(
                                        probs[:, off : off + ln],
                                        praw[:, off : off + ln],
                                        pattern=pattern,
                                        compare_op=ALU.is_ge,
                                        fill=zero_fill,
                                        base=base,
                                        channel_multiplier=cm,
                                    )
                                probs_t[i] = probs

                            def emit_av(i):
                                T, off, ln, _ = tiles[i]
                                probs = probs_t.pop(i)
                                vt = v_s[T // 4]
                                tl = (T % 4) * 256
                                first = i == 0
                                last = i == len(tiles) - 1
                                nc.tensor.matmul(
                                    psm[:, off : off + ln],
                                    onec[:],
                                    probs[:, off : off + ln],
                                    start=first,
                                    stop=last,
                                    skip_group_check=True,
                                )
                                nc.tensor.matmul(
                                    po0[:, off : off + ln],
                                    vt[:, tl : tl + 128],
                                    probs[:, off : off + ln],
                                    start=first,
                                    stop=last,
                                    skip_group_check=True,
                                )
                                nc.tensor.matmul(
                                    po1[:, off : off + ln],
                                    vt[:, tl + 128 : tl + 256],
                                    probs[:, off : off + ln],
                                    start=first,
                                    stop=last,
                                    skip_group_check=True,
                                )

                            # software pipeline: AV for tile i trails the
                            # scores for tile i+2 so PE never waits on the
                            # ACT exp / GpSimd mask chain; the previous
                            # q-tile's normalize chain is emitted after the
                            # first scores block for the same reason.
                            for i in range(len(tiles)):
                                emit_scores(i)
                                if i == 0 and prev_norm is not None:
                                    prev_norm()
                                if i >= 3:
                                    emit_av(i - 3)
                            emit_av(len(tiles) - 3)
                            emit_av(len(tiles) - 2)
                            emit_av(len(tiles) - 1)
                            # free the single psm bank ASAP (ACT copy); the
                            # rest of the normalize chain is deferred into
                            # the next q-tile's score stream.
                            psmb = pC.tile([1, 512], BF16, name="psmb", bufs=2)
                            nc.scalar.copy(psmb[:], psm[:])

                            def make_norm(stream, Q, po0, po1, psmb):
                                def norm():
                                    pbcC = psC.tile(
                                        [128, 512], F32, name="pbcC", bufs=1
                                    )
                                    nc.tensor.matmul(
                                        pbcC[:], oner[:], psmb[:], start=True, stop=True
                                    )
                                    # 1/x as exp(-ln(x)), both on ACT
                                    lnC = pC.tile([128, 512], F32, name="lnC", bufs=2)
                                    nc.scalar.activation(lnC[:], pbcC[:], AF.Ln)
                                    bcsC = pC.tile([128, 512], F32, name="bcsC", bufs=2)
                                    nc.scalar.activation(
                                        bcsC[:], lnC[:], AF.Exp, scale=-1.0
                                    )
                                    for dc in range(2):
                                        po = po0 if dc == 0 else po1
                                        nc.vector.tensor_mul(
                                            outC[stream][dc][:, Q * 512 : (Q + 1) * 512],
                                            po[:],
                                            bcsC[:],
                                        )

                                return norm

                            prev_norm = make_norm(stream, Q, po0, po1, psmb)
                    prev_norm()

                # ================= PHASE D: output projection ============
                with tc.tile_pool(name="pDps", bufs=1, space="PSUM") as psD:
                    # outC is stored contiguously in query order; the lhsT
                    # for contraction chunk (j, dc) is the stride-8 view
                    # q = 8m + j over rows m0..m0+128. m-block-major with
                    # copies deferred one block so only the last block's
                    # copy+DMA is exposed.
                    prev_copy = None
                    for co in range(0, HID, 512):
                        for m in range(4):
                            stream, m0 = m // 2, (m % 2) * 128
                            pos = psD.tile([128, 512], F32, name=f"pD{m}", bufs=2)
                            for kc in range(16):
                                j, dc = kc // 2, kc % 2
                                lhsT = (
                                    outC[stream][dc][:]
                                    .rearrange("p (m j) -> p m j", j=8)
                                    [:, m0 : m0 + 128, j : j + 1]
                                )
                                nc.tensor.matmul(
                                    pos[:],
                                    lhsT,
                                    wo_sb[:, kc * HID + co : kc * HID + co + 512],
                                    start=(kc == 0),
                                    stop=(kc == 15),
                                )

                            def make_copy(co, m, pos):
                                last = co == HID - 512 and m == 3

                                def docopy():
                                    if last:
                                        for hh in range(2):
                                            osth = pC.tile(
                                                [128, 256], F32, name="osth", bufs=2
                                            )
                                            nc.scalar.copy(
                                                osth[:],
                                                pos[:, hh * 256 : (hh + 1) * 256],
                                            )
                                            nc.sync.dma_start(
                                                out_d[
                                                    m * 128 : (m + 1) * 128,
                                                    co + hh * 256 : co + (hh + 1) * 256,
                                                ],
                                                osth[:],
                                            )
                                    else:
                                        ost = pC.tile(
                                            [128, 512], F32, name="ost", bufs=5
                                        )
                                        nc.scalar.copy(ost[:], pos[:])
                                        nc.sync.dma_start(
                                            out_d[
                                                m * 128 : (m + 1) * 128, co : co + 512
                                            ],
                                            ost[:],
                                        )

                                return docopy

                            if prev_copy is not None:
                                prev_copy()
                            prev_copy = make_copy(co, m, pos)
                    prev_copy()
            pCt.__exit__(None, None, None)
    return nc


def _host_inputs(hidden_states, wq, wk, wv, wo, q_norm_w, k_norm_w):
    """Build the 8 per-core input maps (all host-side numpy prep).
    Every tensor is prepacked into its exact SBUF layout so device DMAs
    are plain contiguous copies."""
    hs = np.asarray(hidden_states, dtype=np.float32)
    wq = np.asarray(wq, dtype=np.float32)
    wk = np.asarray(wk, dtype=np.float32)
    wv = np.asarray(wv, dtype=np.float32)
    wo = np.asarray(wo, dtype=np.float32)
    qnw = np.asarray(q_norm_w, dtype=np.float32)
    knw = np.asarray(k_norm_w, dtype=np.float32)

    def pack_w(w):  # [HID, width] -> [128, NKC*width] chunk-major free axis
        width = w.shape[1]
        return np.ascontiguousarray(
            w.reshape(NKC, 128, width).transpose(1, 0, 2).reshape(128, NKC * width)
        ).astype(NPBF)

    # hsT packed per s-tile: [128, (st, kc, 256)]
    hsp = []
    for b in range(B):
        h = hs[b].T.reshape(NKC, 128, 4, 512).transpose(1, 2, 0, 3)
        hsp.append(np.ascontiguousarray(h.reshape(128, 4 * NKC * 512)).astype(NPBF))

    inv_freq = 1.0 / (THETA ** (np.arange(0, D, 2, dtype=np.float32) / D))
    ang = np.outer(inv_freq, np.arange(S, dtype=np.float32))  # (128, S)
    cosp = np.ascontiguousarray(np.cos(ang)).astype(NPBF)
    sinp = np.ascontiguousarray(np.sin(ang)).astype(NPBF)

    invq = np.ascontiguousarray(((1.0 + qnw) ** -2).reshape(2, 128).T).astype(NPBF)
    invk = np.ascontiguousarray(((1.0 + knw) ** -2).reshape(2, 128).T).astype(NPBF)
    onec = np.ones((128, 1), NPBF)
    oner = np.ones((1, 128), NPBF)
    epsb = np.full((1, 1), EPS, np.float32)

    # wo packed: [128, (kc, HID)]
    wop = np.ascontiguousarray(
        wo.reshape(16, 128, HID).transpose(1, 0, 2).reshape(128, 16 * HID)
    ).astype(NPBF)

    qs = 1.0 + qnw
    ks = 1.0 + knw
    in_maps = []
    for core in range(8):
        b, h = core // 4, core % 4
        wq2 = np.concatenate(
            [
                wq[:, h * D : (h + 1) * D] * qs[None, :],
                wq[:, (4 + h) * D : (5 + h) * D] * qs[None, :],
            ],
            axis=1,
        )  # [HID, 512]
        wqp_ = np.stack([pack_w(wq2[:, hc * 128 : (hc + 1) * 128]) for hc in range(4)])
        wk1 = wk[:, h * D : (h + 1) * D] * ks[None, :]
        wkp_ = np.stack([pack_w(wk1[:, hc * 128 : (hc + 1) * 128]) for hc in range(2)])
        wvp_ = pack_w(wv[:, h * D : (h + 1) * D])
        in_maps.append(
            {
                "hsp": hsp[b],
                "wqp": wqp_,
                "wkp": wkp_,
                "wvp": wvp_,
                "wop": wop,
                "cosp": cosp,
                "sinp": sinp,
                "invq": invq,
                "invk": invk,
                "onec": onec,
                "oner": oner,
                "epsb": epsb,
            }
        )
    return in_maps


_PROGRAM = None


def kernel(hidden_states, wq, wk, wv, wo, q_norm_w, k_norm_w):
    global _PROGRAM
    from concourse.bass_utils import run_bass_kernel_spmd

    if _PROGRAM is None:
        _PROGRAM = build_program()
    in_maps = _host_inputs(hidden_states, wq, wk, wv, wo, q_norm_w, k_norm_w)
    res = run_bass_kernel_spmd(_PROGRAM, in_maps, core_ids=list(range(8)))
    out = np.empty((B, S, HID), np.float32)
    for core in range(8):
        b, h = core // 4, core % 4
        out[b, h * 512 : (h + 1) * 512, :] = res.results[core]["out"]
    return out


# revision 35
# speedup vs baseline: 1.1919x; 1.0178x over previous
"""LyraGemma3 sliding-window attention — Trainium2 Bass kernel, 8 NeuronCores.

Sharding: core = b*4 + h  (b in {0,1} batch, h in {0..3} head-group).
Each core owns vanilla head h, lyra head 4+h, kv head h for batch b and
produces output rows [512h, 512h+512) of batch b — disjoint slices, no
collectives.

v4 design:
- All matmul operands bf16; q/k/v and outC SBUF-resident (no DRAM spill).
- Persistent intermediates split per 512-token group so phase C's first
  q-tiles depend only on early phase-A groups (no whole-tile false deps).
- Masks applied multiplicatively after exp via GpSimd affine_select, and
  masked tiles compute only their valid query subrange (causal tiles
  shrink, window-edge tiles grow), with the T loop ordered so a
  full-range tile carries the PSUM-zeroing start flag.
- 1/x and 1/sqrt(x) computed as exp(-ln(x)) / exp(-0.5 ln(x)) on the
  Scalar engine: every activation (Square/Copy/Ln/Exp) lives in one ACT
  table, and the slow DVE reciprocal disappears from all critical paths.
- Phase A/C tails (rstd broadcast + rope, softmax normalize) are
  deferred into the next tile's instruction stream so PE never waits.
- wo streams into SBUF during phase C (address space reused from the
  phase-A weights); phase D runs m-block-major with deferred PSUM->SBUF
  copies so only the last output block's copy+DMA is exposed.
"""

import sys

sys.path.insert(0, "/opt/trn_rl_repo")

import numpy as np
import ml_dtypes

import concourse.bass as bass
import concourse.tile as tile
from concourse import mybir
from concourse.tile import ScopedClock

F32 = mybir.dt.float32
BF16 = mybir.dt.bfloat16
AF = mybir.ActivationFunctionType
ALU = mybir.AluOpType
NPBF = ml_dtypes.bfloat16

B, S, HID = 2, 2048, 2560
H, KV, D = 8, 4, 256
WINDOW = 1024
THETA = 10000.0
EPS = 1e-6
SCALING = 256.0 ** (-0.5)  # 1/16

NKC = HID // 128  # 20 contraction chunks for projections
NST = 8           # phase-A s-tiles of 256 tokens
NT = S // 128     # 16 key tiles of 128
NQ = 4            # attention q-tiles of 512


class SplitWaitTC(tile.TileContext):
    """This container's walrus encodes at most ONE semaphore wait per
    instruction; Tile emits multi-wait sync_info. Hoist extra waits onto
    preceding same-engine NOPs."""

    def _drain_and_barrier(self, tick_clock, wait_clock):
        nc = self.nc
        drain_inst = nc.sync.drain()
        wait_clock.add_sem_waits(
            drain_inst.ins, ScopedClock({None: tick_clock.global_clock})
        )
        self._split_multi_waits()
        nc.all_engine_barrier()
        popped = nc._tile_sem_poison_stack.pop()
        assert popped is self._sem_poison
        nc.clear_and_free_semaphores(list(self.sems.allocated().values()))
        nc.all_engine_barrier()

    def _split_multi_waits(self):
        nc = self.nc
        cur_bb = nc.cur_bb
        assert cur_bb is not None
        for f in nc.m.functions:
            for blk in f.blocks:
                insts = blk.instructions
                i = 0
                while i < len(insts):
                    inst = insts[i]
                    si = inst.sync_info
                    if si is not None and si.on_wait and len(si.on_wait) > 1:
                        waits = list(si.on_wait)
                        inst.sync_info = mybir.SyncInfo(
                            on_wait=waits[-1:], on_update=si.on_update
                        )
                        eng = inst.engine
                        for w in waits[:-1]:
                            nop = nc.engines[eng].nop()
                            nop.ins.sync_info = mybir.SyncInfo(
                                on_wait=[w], on_update=[]
                            )
                            cur_bb.bb.instructions.remove(nop.ins)
                            insts.insert(i, nop.ins)
                            i += 1
                    i += 1


def _c_tiles(Q):
    """Key-tile schedule for q-tile Q (queries [512Q, 512Q+512)).
    Returns [(T, off, length, select)] where [off, off+length) is the
    valid query subrange and select is None or (pattern, base, chan_mult)
    for the post-exp GpSimd affine_select. Ordered so the first entry is
    full-range (its matmul carries start=True and zeroes the whole PSUM
    region)."""
    out = []
    for T in range(max(0, 4 * Q - 4), 4 * Q):  # fully-valid tiles
        out.append((T, 0, 512, None))
    for j in range(4):  # causal diagonal: queries f >= 128 j are live
        ln = 512 - 128 * j
        # keep where f' - p >= 0 (f' is the index within the subrange)
        out.append((4 * Q + j, 128 * j, ln, ([[1, ln]], 0, -1)))
    if Q >= 2:
        for jp in range(4):  # window edge: queries f <= 128 jp + 126 live
            ln = 128 * jp + 128
            # keep where p - f + (128 jp - 1) >= 0
            out.append((4 * Q - 8 + jp, 0, ln, ([[-1, ln]], 128 * jp - 1, 1)))
    return out


def build_program():
    nc = bass.Bass()

    hsp = nc.declare_dram_parameter("hsp", [128, NST * NKC * 256], BF16, isOutput=False)
    wqp = nc.declare_dram_parameter("wqp", [4, 128, NKC * 128], BF16, isOutput=False)
    wkp = nc.declare_dram_parameter("wkp", [2, 128, NKC * 128], BF16, isOutput=False)
    wvp = nc.declare_dram_parameter("wvp", [128, NKC * 256], BF16, isOutput=False)
    wop = nc.declare_dram_parameter("wop", [128, 16 * HID], BF16, isOutput=False)
    cosp = nc.declare_dram_parameter("cosp", [128, S], BF16, isOutput=False)
    sinp = nc.declare_dram_parameter("sinp", [128, S], BF16, isOutput=False)
    invq_d = nc.declare_dram_parameter("invq", [128, 2], BF16, isOutput=False)
    invk_d = nc.declare_dram_parameter("invk", [128, 2], BF16, isOutput=False)
    onec_d = nc.declare_dram_parameter("onec", [128, 1], BF16, isOutput=False)
    oner_d = nc.declare_dram_parameter("oner", [1, 128], BF16, isOutput=False)
    epsb_d = nc.declare_dram_parameter("epsb", [1, 1], F32, isOutput=False)
    out_d = nc.declare_dram_parameter("out", [512, HID], F32, isOutput=True)

    with SplitWaitTC(nc) as tc:
        with tc.tile_pool(name="outer", bufs=1) as pO:
            # persistent intermediates, split per 512-token group g:
            # qTs[g]: [128, c*512 + t], c in {van d0, van d1, lyra d0, lyra d1}
            # kTrs/kTns[g]: [128, c*512 + t], c = d half
            # v_s[g]: [128, tloc*256 + d], tloc = key-tile within group
            qTs = [pO.tile([128, 4 * 512], BF16, name=f"qT{g}") for g in range(4)]
            kTrs = [pO.tile([128, 2 * 512], BF16, name=f"kTr{g}") for g in range(4)]
            kTns = [pO.tile([128, 2 * 512], BF16, name=f"kTn{g}") for g in range(4)]
            v_s = [pO.tile([128, 4 * 256], BF16, name=f"v{g}") for g in range(4)]
            outC = [
                [pO.tile([128, S], BF16, name=f"outC{s}{c}") for c in range(2)]
                for s in range(2)
            ]

            zero_fill = nc.gpsimd.to_reg(0.0)
            # Warm up the GpSimd engine: the first affine_select pays a
            # ~8us ucode-load cost; issue a dummy one before phase A so
            # phase C's first masked tile doesn't eat it.
            warm = pO.tile([128, 8], BF16, name="warm")
            nc.gpsimd.memset(warm[:], 0.0)
            nc.gpsimd.affine_select(
                warm[:], warm[:],
                pattern=[[1, 8]],
                compare_op=ALU.is_ge,
                fill=zero_fill,
                base=0,
                channel_multiplier=-1,
            )

            # Phase C/D SBUF temps live in a pool opened BEFORE phase A so
            # their addresses never overlap phase-A tiles: C's first ops
            # don't wait for the phase-A tail to release its space.
            pCt = tc.tile_pool(name="pCt", bufs=1)
            pC = pCt.__enter__()

            # ================= PHASE A: projections + norm + rope ========
            # 512-token s-tiles (4 of them): half the matmul / copy / rope
            # instruction count of a 256-token pipeline at the same FLOPs.
            with (
                tc.tile_pool(name="pA", bufs=1) as pA,
                tc.tile_pool(name="pAps", bufs=1, space="PSUM") as psA,
            ):
                # hidden states arrive as four quarter tiles per s-tile:
                # the first matmuls start after ~1.3MB of DMA, and the
                # quarters double-buffer across s-tiles.
                hstq_t = {}

                def hst_dma(g):
                    qs = [
                        pA.tile([128, 5 * 512], BF16, name=f"hstq{i}", bufs=2)
                        for i in range(4)
                    ]
                    for i in range(4):
                        nc.sync.dma_start(
                            qs[i][:],
                            hsp[:, (g * 4 + i) * 5 * 512 : (g * 4 + i + 1) * 5 * 512],
                        )
                    hstq_t[g] = qs

                wq_sb = [pA.tile([128, NKC * 128], BF16, name=f"wq{hc}") for hc in range(4)]
                qs0 = [
                    pA.tile([128, 5 * 512], BF16, name=f"hstq{i}", bufs=2)
                    for i in range(4)
                ]
                hstq_t[0] = qs0
                nc.sync.dma_start(qs0[0][:], hsp[:, 0 : 5 * 512])
                nc.sync.dma_start(wq_sb[0][:, 0 : 10 * 128], wqp[0][:, 0 : 10 * 128])
                nc.sync.dma_start(wq_sb[1][:, 0 : 10 * 128], wqp[1][:, 0 : 10 * 128])
                nc.sync.dma_start(qs0[1][:], hsp[:, 5 * 512 : 10 * 512])
                nc.sync.dma_start(
                    wq_sb[0][:, 10 * 128 : NKC * 128], wqp[0][:, 10 * 128 : NKC * 128]
                )
                nc.sync.dma_start(
                    wq_sb[1][:, 10 * 128 : NKC * 128], wqp[1][:, 10 * 128 : NKC * 128]
                )
                for i in range(2, 4):
                    nc.sync.dma_start(qs0[i][:], hsp[:, i * 5 * 512 : (i + 1) * 5 * 512])
                onec = pO.tile([128, 1], BF16, name="onec")
                nc.sync.dma_start(onec[:], onec_d[:])
                oner = pO.tile([1, 128], BF16, name="oner")
                nc.sync.dma_start(oner[:], oner_d[:])
                epsb = pO.tile([1, 1], F32, name="epsb")
                nc.sync.dma_start(epsb[:], epsb_d[:])
                invq = pO.tile([128, 2], BF16, name="invq")
                nc.sync.dma_start(invq[:], invq_d[:])
                invk = pO.tile([128, 2], BF16, name="invk")
                nc.sync.dma_start(invk[:], invk_d[:])
                for hc in range(2, 4):
                    nc.sync.dma_start(wq_sb[hc][:], wqp[hc])
                wk_sb = [pA.tile([128, NKC * 128], BF16, name=f"wk{hc}") for hc in range(2)]
                for hc in range(2):
                    nc.sync.dma_start(wk_sb[hc][:], wkp[hc])
                wv_sb = pA.tile([128, NKC * 256], BF16, name="wv_sb")
                nc.sync.dma_start(wv_sb[:], wvp[:])
                hst_dma(1)
                cos_sb = pA.tile([128, S], BF16, name="cos_sb")
                nc.sync.dma_start(cos_sb[:], cosp[:])
                sin_sb = pA.tile([128, S], BF16, name="sin_sb")
                nc.sync.dma_start(sin_sb[:], sinp[:])

                def hs_ap(g, kc, lo, width):
                    t = hstq_t[g][kc // 5]
                    base = (kc % 5) * 512
                    return t[:, base + lo : base + lo + width]

                prev_tail = None
                for g in range(4):
                    if g + 2 < 4:
                        hst_dma(g + 2)
                    # ---- projections (accumulate over 20 HID chunks) ----
                    qz = pA.tile([128, 2048], BF16, name="qz", bufs=2)
                    if g == 0:
                        # quarter-paced warm-up: two concurrent accumulation
                        # groups walk the chunks in DMA-arrival order
                        for pair in ((0, 1), (2, 3)):
                            paccs = {
                                hc: psA.tile([128, 512], F32, name="pacc", bufs=2)
                                for hc in pair
                            }
                            for qtr in range(4):
                                for hc in pair:
                                    for kc in range(qtr * 5, qtr * 5 + 5):
                                        nc.tensor.matmul(
                                            paccs[hc][:],
                                            wq_sb[hc][:, kc * 128 : (kc + 1) * 128],
                                            hs_ap(g, kc, 0, 512),
                                            start=(kc == 0),
                                            stop=(kc == NKC - 1),
                                            skip_group_check=True,
                                        )
                            for hc in pair:
                                nc.scalar.copy(
                                    qz[:, hc * 512 : (hc + 1) * 512], paccs[hc][:]
                                )
                    else:
                        for hc in range(4):
                            pq = psA.tile([128, 512], F32, name="pacc", bufs=2)
                            for kc in range(NKC):
                                nc.tensor.matmul(
                                    pq[:],
                                    wq_sb[hc][:, kc * 128 : (kc + 1) * 128],
                                    hs_ap(g, kc, 0, 512),
                                    start=(kc == 0),
                                    stop=(kc == NKC - 1),
                                )
                            nc.scalar.copy(qz[:, hc * 512 : (hc + 1) * 512], pq[:])
                    sqq = pA.tile([128, 2048], BF16, name="sqq", bufs=1)
                    nc.scalar.activation(sqq[:], qz[:], AF.Square)
                    kz = pA.tile([128, 1024], BF16, name="kz", bufs=2)
                    for hc in range(2):
                        pk = psA.tile([128, 512], F32, name="pacc", bufs=2)
                        for kc in range(NKC):
                            nc.tensor.matmul(
                                pk[:],
                                wk_sb[hc][:, kc * 128 : (kc + 1) * 128],
                                hs_ap(g, kc, 0, 512),
                                start=(kc == 0),
                                stop=(kc == NKC - 1),
                            )
                        nc.scalar.copy(kz[:, hc * 512 : (hc + 1) * 512], pk[:])
                    sqk = pA.tile([128, 1024], BF16, name="sqk", bufs=1)
                    nc.scalar.activation(sqk[:], kz[:], AF.Square)

                    def emit_pn(head):
                        pn = psA.tile([1, 512], F32, name="pn", bufs=2)
                        for c in range(2):
                            if head < 2:
                                rhs = sqq[:, (head * 2 + c) * 512 : (head * 2 + c + 1) * 512]
                                lhsT = invq[:, c : c + 1]
                            else:
                                rhs = sqk[:, c * 512 : (c + 1) * 512]
                                lhsT = invk[:, c : c + 1]
                            nc.tensor.matmul(
                                pn[:], lhsT, rhs, start=(c == 0), stop=(c == 1)
                            )
                        lnm = pA.tile([1, 512], F32, name="lnm", bufs=2)
                        nc.scalar.activation(
                            lnm[:], pn[:], AF.Ln, bias=epsb[:], scale=1.0 / 256.0
                        )
                        rst = pA.tile([1, 512], BF16, name="rst", bufs=2)
                        nc.scalar.activation(rst[:], lnm[:], AF.Exp, scale=-0.5)
                        return rst

                    rsts = [emit_pn(0), emit_pn(1)]
                    for sm in range(4):
                        pv = psA.tile([128, 256], F32, name="pvacc", bufs=2)
                        for kc in range(NKC):
                            nc.tensor.matmul(
                                pv[:],
                                hs_ap(g, kc, sm * 128, 128),
                                wv_sb[:, kc * 256 : (kc + 1) * 256],
                                start=(kc == 0),
                                stop=(kc == NKC - 1),
                            )
                        nc.scalar.copy(
                            v_s[g][:, sm * 256 : sm * 256 + 256], pv[:]
                        )

                    rsts.append(emit_pn(2))

                    # tail (rstd broadcast + rope) for the PREVIOUS s-tile:
                    # its norm chain has finished, so the pbc matmuls never
                    # stall PE, and rope (DVE) runs under this tile's
                    # projections.
                    def make_tail(g, qz, kz, rsts):
                        s0 = g * 512

                        def tail():
                            bcs = []
                            for head in range(3):
                                pbc = psA.tile([128, 512], F32, name="pbc", bufs=2)
                                nc.tensor.matmul(
                                    pbc[:], oner[:], rsts[head][:], start=True, stop=True
                                )
                                bc = pA.tile([128, 512], BF16, name=f"bc{head}", bufs=1)
                                nc.vector.tensor_copy(bc[:], pbc[:])
                                bcs.append(bc)
                            cs = cos_sb[:, s0 : s0 + 512]
                            sn = sin_sb[:, s0 : s0 + 512]

                            def rope2(z0, z1, bc, d0, d1):
                                t0 = pA.tile([128, 512], BF16, name="t0", bufs=1)
                                nc.vector.tensor_mul(t0[:], z0, cs)
                                t1 = pA.tile([128, 512], BF16, name="t1", bufs=1)
                                nc.vector.tensor_mul(t1[:], z1, sn)
                                u0 = pA.tile([128, 512], BF16, name="u0", bufs=1)
                                nc.vector.tensor_sub(u0[:], t0[:], t1[:])
                                nc.vector.tensor_mul(d0, u0[:], bc[:])
                                t2 = pA.tile([128, 512], BF16, name="t2", bufs=1)
                                nc.vector.tensor_mul(t2[:], z1, cs)
                                t3 = pA.tile([128, 512], BF16, name="t3", bufs=1)
                                nc.vector.tensor_mul(t3[:], z0, sn)
                                u1 = pA.tile([128, 512], BF16, name="u1", bufs=1)
                                nc.vector.tensor_add(u1[:], t2[:], t3[:])
                                nc.vector.tensor_mul(d1, u1[:], bc[:])

                            for head in range(2):
                                rope2(
                                    qz[:, (head * 2) * 512 : (head * 2) * 512 + 512],
                                    qz[:, (head * 2 + 1) * 512 : (head * 2 + 1) * 512 + 512],
                                    bcs[head],
                                    qTs[g][:, (head * 2) * 512 : (head * 2) * 512 + 512],
                                    qTs[g][:, (head * 2 + 1) * 512 : (head * 2 + 1) * 512 + 512],
                                )
                            rope2(
                                kz[:, 0:512], kz[:, 512:1024], bcs[2],
                                kTrs[g][:, 0:512],
                                kTrs[g][:, 512:1024],
                            )
                            nc.vector.tensor_mul(
                                kTns[g][:, 0:512], kz[:, 0:512], bcs[2][:]
                            )
                            nc.vector.tensor_mul(
                                kTns[g][:, 512:1024], kz[:, 512:1024], bcs[2][:]
                            )

                        return tail

                    if prev_tail is not None:
                        prev_tail()
                    prev_tail = make_tail(g, qz, kz, rsts)
                prev_tail()

            # ================= PHASES C+D ================================
            # wo streams into the SBUF space the phase-A pool released.
            with tc.tile_pool(name="pWo", bufs=1) as pWo:
                wo_sb = pWo.tile([128, 16 * HID], BF16, name="wo_sb")
                for i in range(4):
                    nc.sync.dma_start(
                        wo_sb[:, i * 4 * HID : (i + 1) * 4 * HID],
                        wop[:, i * 4 * HID : (i + 1) * 4 * HID],
                    )

                # ================= PHASE C: attention ====================
                with tc.tile_pool(name="pCps", bufs=1, space="PSUM") as psC:
                    # Pre-create every psum name in an order that maps the
                    # first q-tile's tiles (pss A/B, psm, po0 A, po1 A) onto
                    # phase-A banks whose last readers finish mid-tile
                    # (pacc/pvacc/pn, read by ACT), while the banks held
                    # until the phase-A tail's DVE reads (pbc) are only
                    # reused from the second q-tile on (po0/po1 buffer B)
                    # or by pbcC (first used at the first normalize).
                    for _ in range(2):
                        psC.tile([128, 512], F32, name="pss", bufs=2)
                    psC.tile([1, 512], F32, name="psm", bufs=1)
                    psC.tile([128, 512], F32, name="po0", bufs=2)
                    psC.tile([128, 512], F32, name="po1", bufs=2)
                    psC.tile([128, 512], F32, name="po0", bufs=2)
                    psC.tile([128, 512], F32, name="po1", bufs=2)
                    psC.tile([128, 512], F32, name="pbcC", bufs=1)
                    prev_norm = None
                    for stream in range(2):  # 0 = vanilla (roped k), 1 = lyra
                        kTg = kTrs if stream == 0 else kTns
                        # Q0 is all causal-masked tiles; run it last so the
                        # exp->select chain is warm and pipelined when it hits.
                        for Q in (1, 2, 3, 0):
                            tiles = _c_tiles(Q)
                            po0 = psC.tile([128, 512], F32, name="po0", bufs=2)
                            po1 = psC.tile([128, 512], F32, name="po1", bufs=2)
                            psm = psC.tile([1, 512], F32, name="psm", bufs=1)
                            probs_t = {}

                            def emit_scores(i):
                                T, off, ln, sel = tiles[i]
                                kT = kTg[T // 4]
                                tl = (T % 4) * 128
                                pss = psC.tile([128, 512], F32, name="pss", bufs=2)
                                for c in range(2):
                                    qb = (2 * stream + c) * 512 + off
                                    nc.tensor.matmul(
                                        pss[:, off : off + ln],
                                        kT[:, c * 512 + tl : c * 512 + tl + 128],
                                        qTs[Q][:, qb : qb + ln],
                                        start=(c == 0),
                                        stop=(c == 1),
                                    )
                                probs = pC.tile([128, 512], BF16, name="probs", bufs=5)
                                if sel is None:
                                    nc.scalar.activation(
                                        probs[:, off : off + ln],
                                        pss[:, off : off + ln],
                                        AF.Exp,
                                        scale=SCALING,
                                    )
                                else:
                                    pattern, base, cm = sel
                                    praw = pC.tile([128, 512], BF16, name="praw", bufs=2)
                                    nc.scalar.activation(
                                        praw[:, off : off + ln],
                                        pss[:, off : off + ln],
                                        AF.Exp,
                                        scale=SCALING,
                                    )
                                    nc.gpsimd.affine_select(
                                        probs[:, off : off + ln],
                                        praw[:, off : off + ln],
                                        pattern=pattern,
                                        compare_op=ALU.is_ge,
                                        fill=zero_fill,
                                        base=base,
                                        channel_multiplier=cm,
                                    )
                                probs_t[i] = probs

                            def emit_av(i):
                                T, off, ln, _ = tiles[i]
                                probs = probs_t.pop(i)
                                vt = v_s[T // 4]
                                tl = (T % 4) * 256
                                first = i == 0
                                last = i == len(tiles) - 1
                                nc.tensor.matmul(
                                    psm[:, off : off + ln],
                                    onec[:],
                                    probs[:, off : off + ln],
                                    start=first,
                                    stop=last,
                                    skip_group_check=True,
                                )
                                nc.tensor.matmul(
                                    po0[:, off : off + ln],
                                    vt[:, tl : tl + 128],
                                    probs[:, off : off + ln],
                                    start=first,
                                    stop=last,
                                    skip_group_check=True,
                                )
                                nc.tensor.matmul(
                                    po1[:, off : off + ln],
                                    vt[:, tl + 128 : tl + 256],
                                    probs[:, off : off + ln],
                                    start=first,
                                    stop=last,
                                    skip_group_check=True,
                                )

                            # software pipeline: AV for tile i trails the
                            # scores for tile i+2 so PE never waits on the
                            # ACT exp / GpSimd mask chain; the previous
                            # q-tile's normalize chain is emitted after the
                            # first scores block for the same reason.
                            for i in range(len(tiles)):
                                emit_scores(i)
                                if i == 0 and prev_norm is not None:
                                    prev_norm()
                                if i >= 3:
                                    emit_av(i - 3)
                            emit_av(len(tiles) - 3)
                            emit_av(len(tiles) - 2)
                            emit_av(len(tiles) - 1)
                            # free the single psm bank ASAP (ACT copy); the
                            # rest of the normalize chain is deferred into
                            # the next q-tile's score stream.
                            psmb = pC.tile([1, 512], BF16, name="psmb", bufs=2)
                            nc.scalar.copy(psmb[:], psm[:])

                            def make_norm(stream, Q, po0, po1, psmb):
                                def norm():
                                    pbcC = psC.tile(
                                        [128, 512], F32, name="pbcC", bufs=1
                                    )
                                    nc.tensor.matmul(
                                        pbcC[:], oner[:], psmb[:], start=True, stop=True
                                    )
                                    # 1/x as exp(-ln(x)), both on ACT
                                    lnC = pC.tile([128, 512], F32, name="lnC", bufs=2)
                                    nc.scalar.activation(lnC[:], pbcC[:], AF.Ln)
                                    bcsC = pC.tile([128, 512], F32, name="bcsC", bufs=2)
                                    nc.scalar.activation(
                                        bcsC[:], lnC[:], AF.Exp, scale=-1.0
                                    )
                                    for dc in range(2):
                                        po = po0 if dc == 0 else po1
                                        nc.vector.tensor_mul(
                                            outC[stream][dc][:, Q * 512 : (Q + 1) * 512],
                                            po[:],
                                            bcsC[:],
                                        )

                                return norm

                            prev_norm = make_norm(stream, Q, po0, po1, psmb)
                    prev_norm()

                # ================= PHASE D: output projection ============
                with tc.tile_pool(name="pDps", bufs=1, space="PSUM") as psD:
                    # outC is stored contiguously in query order; the lhsT
                    # for contraction chunk (j, dc) is the stride-8 view
                    # q = 8m + j over rows m0..m0+128. m-block-major with
                    # copies deferred one block so only the last block's
                    # copy+DMA is exposed.
                    prev_copy = None
                    for co in range(0, HID, 512):
                        for m in range(4):
                            stream, m0 = m // 2, (m % 2) * 128
                            pos = psD.tile([128, 512], F32, name=f"pD{m}", bufs=2)
                            final = co == HID - 512 and m == 3
                            if final:
                                # last block: two half-width accumulations so
                                # the first half's copy+DMA hides under the
                                # second half's matmuls, shrinking the drain.
                                if prev_copy is not None:
                                    prev_copy()
                                    prev_copy = None
                                for hh in range(2):
                                    for kc in range(16):
                                        j, dc = kc // 2, kc % 2
                                        lhsT = (
                                            outC[stream][dc][:]
                                            .rearrange("p (m j) -> p m j", j=8)
                                            [:, m0 : m0 + 128, j : j + 1]
                                        )
                                        nc.tensor.matmul(
                                            pos[:, hh * 256 : (hh + 1) * 256],
                                            lhsT,
                                            wo_sb[
                                                :,
                                                kc * HID + co + hh * 256 : kc * HID
                                                + co
                                                + (hh + 1) * 256,
                                            ],
                                            start=(kc == 0),
                                            stop=(kc == 15),
                                            skip_group_check=True,
                                        )
                                    osth = pC.tile(
                                        [128, 256], F32, name="osth", bufs=2
                                    )
                                    nc.scalar.copy(
                                        osth[:], pos[:, hh * 256 : (hh + 1) * 256]
                                    )
                                    nc.sync.dma_start(
                                        out_d[
                                            m * 128 : (m + 1) * 128,
                                            co + hh * 256 : co + (hh + 1) * 256,
                                        ],
                                        osth[:],
                                    )
                                continue
                            for kc in range(16):
                                j, dc = kc // 2, kc % 2
                                lhsT = (
                                    outC[stream][dc][:]
                                    .rearrange("p (m j) -> p m j", j=8)
                                    [:, m0 : m0 + 128, j : j + 1]
                                )
                                nc.tensor.matmul(
                                    pos[:],
                                    lhsT,
                                    wo_sb[:, kc * HID + co : kc * HID + co + 512],
                                    start=(kc == 0),
                                    stop=(kc == 15),
                                )

                            def make_copy(co, m, pos):
                                def docopy():
                                    ost = pC.tile(
                                        [128, 512], F32, name="ost", bufs=5
                                    )
                                    nc.scalar.copy(ost[:], pos[:])
                                    nc.sync.dma_start(
                                        out_d[
                                            m * 128 : (m + 1) * 128, co : co + 512
                                        ],
                                        ost[:],
                                    )

                                return docopy

                            if prev_copy is not None:
                                prev_copy()
                            prev_copy = make_copy(co, m, pos)
                    if prev_copy is not None:
                        prev_copy()
            pCt.__exit__(None, None, None)
    return nc


def _host_inputs(hidden_states, wq, wk, wv, wo, q_norm_w, k_norm_w):
    """Build the 8 per-core input maps (all host-side numpy prep).
    Every tensor is prepacked into its exact SBUF layout so device DMAs
    are plain contiguous copies."""
    hs = np.asarray(hidden_states, dtype=np.float32)
    wq = np.asarray(wq, dtype=np.float32)
    wk = np.asarray(wk, dtype=np.float32)
    wv = np.asarray(wv, dtype=np.float32)
    wo = np.asarray(wo, dtype=np.float32)
    qnw = np.asarray(q_norm_w, dtype=np.float32)
    knw = np.asarray(k_norm_w, dtype=np.float32)

    def pack_w(w):  # [HID, width] -> [128, NKC*width] chunk-major free axis
        width = w.shape[1]
        return np.ascontiguousarray(
            w.reshape(NKC, 128, width).transpose(1, 0, 2).reshape(128, NKC * width)
        ).astype(NPBF)

    # hsT packed per s-tile: [128, (st, kc, 256)]
    hsp = []
    for b in range(B):
        h = hs[b].T.reshape(NKC, 128, 4, 512).transpose(1, 2, 0, 3)
        hsp.append(np.ascontiguousarray(h.reshape(128, 4 * NKC * 512)).astype(NPBF))

    inv_freq = 1.0 / (THETA ** (np.arange(0, D, 2, dtype=np.float32) / D))
    ang = np.outer(inv_freq, np.arange(S, dtype=np.float32))  # (128, S)
    cosp = np.ascontiguousarray(np.cos(ang)).astype(NPBF)
    sinp = np.ascontiguousarray(np.sin(ang)).astype(NPBF)

    invq = np.ascontiguousarray(((1.0 + qnw) ** -2).reshape(2, 128).T).astype(NPBF)
    invk = np.ascontiguousarray(((1.0 + knw) ** -2).reshape(2, 128).T).astype(NPBF)
    onec = np.ones((128, 1), NPBF)
    oner = np.ones((1, 128), NPBF)
    epsb = np.full((1, 1), EPS, np.float32)

    # wo packed: [128, (kc, HID)]
    wop = np.ascontiguousarray(
        wo.reshape(16, 128, HID).transpose(1, 0, 2).reshape(128, 16 * HID)
    ).astype(NPBF)

    qs = 1.0 + qnw
    ks = 1.0 + knw
    in_maps = []
    for core in range(8):
        b, h = core // 4, core % 4
        wq2 = np.concatenate(
            [
                wq[:, h * D : (h + 1) * D] * qs[None, :],
                wq[:, (4 + h) * D : (5 + h) * D] * qs[None, :],
            ],
            axis=1,
        )  # [HID, 512]
        wqp_ = np.stack([pack_w(wq2[:, hc * 128 : (hc + 1) * 128]) for hc in range(4)])
        wk1 = wk[:, h * D : (h + 1) * D] * ks[None, :]
        wkp_ = np.stack([pack_w(wk1[:, hc * 128 : (hc + 1) * 128]) for hc in range(2)])
        wvp_ = pack_w(wv[:, h * D : (h + 1) * D])
        in_maps.append(
            {
                "hsp": hsp[b],
                "wqp": wqp_,
                "wkp": wkp_,
                "wvp": wvp_,
                "wop": wop,
                "cosp": cosp,
                "sinp": sinp,
                "invq": invq,
                "invk": invk,
                "onec": onec,
                "oner": oner,
                "epsb": epsb,
            }
        )
    return in_maps


_PROGRAM = None


def kernel(hidden_states, wq, wk, wv, wo, q_norm_w, k_norm_w):
    global _PROGRAM
    from concourse.bass_utils import run_bass_kernel_spmd

    if _PROGRAM is None:
        _PROGRAM = build_program()
    in_maps = _host_inputs(hidden_states, wq, wk, wv, wo, q_norm_w, k_norm_w)
    res = run_bass_kernel_spmd(_PROGRAM, in_maps, core_ids=list(range(8)))
    out = np.empty((B, S, HID), np.float32)
    for core in range(8):
        b, h = core // 4, core % 4
        out[b, h * 512 : (h + 1) * 512, :] = res.results[core]["out"]
    return out


# revision 36
# speedup vs baseline: 1.1933x; 1.0012x over previous
"""LyraGemma3 sliding-window attention — Trainium2 Bass kernel, 8 NeuronCores.

Sharding: core = b*4 + h  (b in {0,1} batch, h in {0..3} head-group).
Each core owns vanilla head h, lyra head 4+h, kv head h for batch b and
produces output rows [512h, 512h+512) of batch b — disjoint slices, no
collectives.

v4 design:
- All matmul operands bf16; q/k/v and outC SBUF-resident (no DRAM spill).
- Persistent intermediates split per 512-token group so phase C's first
  q-tiles depend only on early phase-A groups (no whole-tile false deps).
- Masks applied multiplicatively after exp via GpSimd affine_select, and
  masked tiles compute only their valid query subrange (causal tiles
  shrink, window-edge tiles grow), with the T loop ordered so a
  full-range tile carries the PSUM-zeroing start flag.
- 1/x and 1/sqrt(x) computed as exp(-ln(x)) / exp(-0.5 ln(x)) on the
  Scalar engine: every activation (Square/Copy/Ln/Exp) lives in one ACT
  table, and the slow DVE reciprocal disappears from all critical paths.
- Phase A/C tails (rstd broadcast + rope, softmax normalize) are
  deferred into the next tile's instruction stream so PE never waits.
- wo streams into SBUF during phase C (address space reused from the
  phase-A weights); phase D runs m-block-major with deferred PSUM->SBUF
  copies so only the last output block's copy+DMA is exposed.
"""

import sys

sys.path.insert(0, "/opt/trn_rl_repo")

import numpy as np
import ml_dtypes

import concourse.bass as bass
import concourse.tile as tile
from concourse import mybir
from concourse.tile import ScopedClock

F32 = mybir.dt.float32
BF16 = mybir.dt.bfloat16
AF = mybir.ActivationFunctionType
ALU = mybir.AluOpType
NPBF = ml_dtypes.bfloat16

B, S, HID = 2, 2048, 2560
H, KV, D = 8, 4, 256
WINDOW = 1024
THETA = 10000.0
EPS = 1e-6
SCALING = 256.0 ** (-0.5)  # 1/16

NKC = HID // 128  # 20 contraction chunks for projections
NST = 8           # phase-A s-tiles of 256 tokens
NT = S // 128     # 16 key tiles of 128
NQ = 4            # attention q-tiles of 512


class SplitWaitTC(tile.TileContext):
    """This container's walrus encodes at most ONE semaphore wait per
    instruction; Tile emits multi-wait sync_info. Hoist extra waits onto
    preceding same-engine NOPs."""

    def _drain_and_barrier(self, tick_clock, wait_clock):
        nc = self.nc
        drain_inst = nc.sync.drain()
        wait_clock.add_sem_waits(
            drain_inst.ins, ScopedClock({None: tick_clock.global_clock})
        )
        self._split_multi_waits()
        nc.all_engine_barrier()
        popped = nc._tile_sem_poison_stack.pop()
        assert popped is self._sem_poison
        nc.clear_and_free_semaphores(list(self.sems.allocated().values()))
        nc.all_engine_barrier()

    def _split_multi_waits(self):
        nc = self.nc
        cur_bb = nc.cur_bb
        assert cur_bb is not None
        for f in nc.m.functions:
            for blk in f.blocks:
                insts = blk.instructions
                i = 0
                while i < len(insts):
                    inst = insts[i]
                    si = inst.sync_info
                    if si is not None and si.on_wait and len(si.on_wait) > 1:
                        waits = list(si.on_wait)
                        inst.sync_info = mybir.SyncInfo(
                            on_wait=waits[-1:], on_update=si.on_update
                        )
                        eng = inst.engine
                        for w in waits[:-1]:
                            nop = nc.engines[eng].nop()
                            nop.ins.sync_info = mybir.SyncInfo(
                                on_wait=[w], on_update=[]
                            )
                            cur_bb.bb.instructions.remove(nop.ins)
                            insts.insert(i, nop.ins)
                            i += 1
                    i += 1


def _c_tiles(Q):
    """Key-tile schedule for q-tile Q (queries [512Q, 512Q+512)).
    Returns [(T, off, length, select)] where [off, off+length) is the
    valid query subrange and select is None or (pattern, base, chan_mult)
    for the post-exp GpSimd affine_select. Ordered so the first entry is
    full-range (its matmul carries start=True and zeroes the whole PSUM
    region)."""
    out = []
    for T in range(max(0, 4 * Q - 4), 4 * Q):  # fully-valid tiles
        out.append((T, 0, 512, None))
    for j in range(4):  # causal diagonal: queries f >= 128 j are live
        ln = 512 - 128 * j
        # keep where f' - p >= 0 (f' is the index within the subrange)
        out.append((4 * Q + j, 128 * j, ln, ([[1, ln]], 0, -1)))
    if Q >= 2:
        for jp in range(4):  # window edge: queries f <= 128 jp + 126 live
            ln = 128 * jp + 128
            # keep where p - f + (128 jp - 1) >= 0
            out.append((4 * Q - 8 + jp, 0, ln, ([[-1, ln]], 128 * jp - 1, 1)))
    return out


def build_program():
    nc = bass.Bass()

    hsp = nc.declare_dram_parameter("hsp", [128, NST * NKC * 256], BF16, isOutput=False)
    wqp = nc.declare_dram_parameter("wqp", [4, 128, NKC * 128], BF16, isOutput=False)
    wkp = nc.declare_dram_parameter("wkp", [2, 128, NKC * 128], BF16, isOutput=False)
    wvp = nc.declare_dram_parameter("wvp", [128, NKC * 256], BF16, isOutput=False)
    wop = nc.declare_dram_parameter("wop", [128, 16 * HID], BF16, isOutput=False)
    cosp = nc.declare_dram_parameter("cosp", [128, S], BF16, isOutput=False)
    sinp = nc.declare_dram_parameter("sinp", [128, S], BF16, isOutput=False)
    invq_d = nc.declare_dram_parameter("invq", [128, 2], BF16, isOutput=False)
    invk_d = nc.declare_dram_parameter("invk", [128, 2], BF16, isOutput=False)
    onec_d = nc.declare_dram_parameter("onec", [128, 1], BF16, isOutput=False)
    oner_d = nc.declare_dram_parameter("oner", [1, 128], BF16, isOutput=False)
    epsb_d = nc.declare_dram_parameter("epsb", [1, 1], F32, isOutput=False)
    out_d = nc.declare_dram_parameter("out", [512, HID], F32, isOutput=True)

    with SplitWaitTC(nc) as tc:
        with tc.tile_pool(name="outer", bufs=1) as pO:
            # persistent intermediates, split per 512-token group g:
            # qTs[g]: [128, c*512 + t], c in {van d0, van d1, lyra d0, lyra d1}
            # kTrs/kTns[g]: [128, c*512 + t], c = d half
            # v_s[g]: [128, tloc*256 + d], tloc = key-tile within group
            qTs = [pO.tile([128, 4 * 512], BF16, name=f"qT{g}") for g in range(4)]
            kTrs = [pO.tile([128, 2 * 512], BF16, name=f"kTr{g}") for g in range(4)]
            kTns = [pO.tile([128, 2 * 512], BF16, name=f"kTn{g}") for g in range(4)]
            v_s = [pO.tile([128, 4 * 256], BF16, name=f"v{g}") for g in range(4)]
            outC = [
                [pO.tile([128, S], BF16, name=f"outC{s}{c}") for c in range(2)]
                for s in range(2)
            ]

            zero_fill = nc.gpsimd.to_reg(0.0)
            # Warm up the GpSimd engine: the first affine_select pays a
            # ~8us ucode-load cost; issue a dummy one before phase A so
            # phase C's first masked tile doesn't eat it.
            warm = pO.tile([128, 8], BF16, name="warm")
            nc.gpsimd.memset(warm[:], 0.0)
            nc.gpsimd.affine_select(
                warm[:], warm[:],
                pattern=[[1, 8]],
                compare_op=ALU.is_ge,
                fill=zero_fill,
                base=0,
                channel_multiplier=-1,
            )

            # Phase C/D SBUF temps live in a pool opened BEFORE phase A so
            # their addresses never overlap phase-A tiles: C's first ops
            # don't wait for the phase-A tail to release its space.
            pCt = tc.tile_pool(name="pCt", bufs=1)
            pC = pCt.__enter__()

            # ================= PHASE A: projections + norm + rope ========
            # 512-token s-tiles (4 of them): half the matmul / copy / rope
            # instruction count of a 256-token pipeline at the same FLOPs.
            with (
                tc.tile_pool(name="pA", bufs=1) as pA,
                tc.tile_pool(name="pAps", bufs=1, space="PSUM") as psA,
            ):
                # hidden states arrive as four quarter tiles per s-tile:
                # the first matmuls start after ~1.3MB of DMA, and the
                # quarters double-buffer across s-tiles.
                hstq_t = {}

                def hst_dma(g):
                    qs = [
                        pA.tile([128, 5 * 512], BF16, name=f"hstq{i}", bufs=2)
                        for i in range(4)
                    ]
                    for i in range(4):
                        nc.sync.dma_start(
                            qs[i][:],
                            hsp[:, (g * 4 + i) * 5 * 512 : (g * 4 + i + 1) * 5 * 512],
                        )
                    hstq_t[g] = qs

                wq_sb = [pA.tile([128, NKC * 128], BF16, name=f"wq{hc}") for hc in range(4)]
                qs0 = [
                    pA.tile([128, 5 * 512], BF16, name=f"hstq{i}", bufs=2)
                    for i in range(4)
                ]
                hstq_t[0] = qs0
                nc.sync.dma_start(qs0[0][:], hsp[:, 0 : 5 * 512])
                nc.sync.dma_start(wq_sb[0][:, 0 : 10 * 128], wqp[0][:, 0 : 10 * 128])
                nc.sync.dma_start(wq_sb[1][:, 0 : 10 * 128], wqp[1][:, 0 : 10 * 128])
                nc.sync.dma_start(qs0[1][:], hsp[:, 5 * 512 : 10 * 512])
                nc.sync.dma_start(
                    wq_sb[0][:, 10 * 128 : NKC * 128], wqp[0][:, 10 * 128 : NKC * 128]
                )
                nc.sync.dma_start(
                    wq_sb[1][:, 10 * 128 : NKC * 128], wqp[1][:, 10 * 128 : NKC * 128]
                )
                for i in range(2, 4):
                    nc.sync.dma_start(qs0[i][:], hsp[:, i * 5 * 512 : (i + 1) * 5 * 512])
                for hc in range(2, 4):
                    nc.sync.dma_start(wq_sb[hc][:], wqp[hc])
                wk_sb = [pA.tile([128, NKC * 128], BF16, name=f"wk{hc}") for hc in range(2)]
                for hc in range(2):
                    nc.sync.dma_start(wk_sb[hc][:], wkp[hc])
                wv_sb = pA.tile([128, NKC * 256], BF16, name="wv_sb")
                nc.sync.dma_start(wv_sb[:], wvp[:])
                hst_dma(1)
                cos_sb = pA.tile([128, S], BF16, name="cos_sb")
                nc.sync.dma_start(cos_sb[:], cosp[:])
                sin_sb = pA.tile([128, S], BF16, name="sin_sb")
                nc.sync.dma_start(sin_sb[:], sinp[:])
                onec = pO.tile([128, 1], BF16, name="onec")
                nc.sync.dma_start(onec[:], onec_d[:])
                oner = pO.tile([1, 128], BF16, name="oner")
                nc.sync.dma_start(oner[:], oner_d[:])
                epsb = pO.tile([1, 1], F32, name="epsb")
                nc.sync.dma_start(epsb[:], epsb_d[:])
                invq = pO.tile([128, 2], BF16, name="invq")
                nc.sync.dma_start(invq[:], invq_d[:])
                invk = pO.tile([128, 2], BF16, name="invk")
                nc.sync.dma_start(invk[:], invk_d[:])

                def hs_ap(g, kc, lo, width):
                    t = hstq_t[g][kc // 5]
                    base = (kc % 5) * 512
                    return t[:, base + lo : base + lo + width]

                prev_tail = None
                for g in range(4):
                    if g + 2 < 4:
                        hst_dma(g + 2)
                    # ---- projections (accumulate over 20 HID chunks) ----
                    qz = pA.tile([128, 2048], BF16, name="qz", bufs=2)
                    if g == 0:
                        # quarter-paced warm-up: two concurrent accumulation
                        # groups walk the chunks in DMA-arrival order
                        for pair in ((0, 1), (2, 3)):
                            paccs = {
                                hc: psA.tile([128, 512], F32, name="pacc", bufs=2)
                                for hc in pair
                            }
                            for qtr in range(4):
                                for hc in pair:
                                    for kc in range(qtr * 5, qtr * 5 + 5):
                                        nc.tensor.matmul(
                                            paccs[hc][:],
                                            wq_sb[hc][:, kc * 128 : (kc + 1) * 128],
                                            hs_ap(g, kc, 0, 512),
                                            start=(kc == 0),
                                            stop=(kc == NKC - 1),
                                            skip_group_check=True,
                                        )
                            for hc in pair:
                                nc.scalar.copy(
                                    qz[:, hc * 512 : (hc + 1) * 512], paccs[hc][:]
                                )
                    else:
                        for hc in range(4):
                            pq = psA.tile([128, 512], F32, name="pacc", bufs=2)
                            for kc in range(NKC):
                                nc.tensor.matmul(
                                    pq[:],
                                    wq_sb[hc][:, kc * 128 : (kc + 1) * 128],
                                    hs_ap(g, kc, 0, 512),
                                    start=(kc == 0),
                                    stop=(kc == NKC - 1),
                                )
                            nc.scalar.copy(qz[:, hc * 512 : (hc + 1) * 512], pq[:])
                    sqq = pA.tile([128, 2048], BF16, name="sqq", bufs=1)
                    nc.scalar.activation(sqq[:], qz[:], AF.Square)
                    kz = pA.tile([128, 1024], BF16, name="kz", bufs=2)
                    for hc in range(2):
                        pk = psA.tile([128, 512], F32, name="pacc", bufs=2)
                        for kc in range(NKC):
                            nc.tensor.matmul(
                                pk[:],
                                wk_sb[hc][:, kc * 128 : (kc + 1) * 128],
                                hs_ap(g, kc, 0, 512),
                                start=(kc == 0),
                                stop=(kc == NKC - 1),
                            )
                        nc.scalar.copy(kz[:, hc * 512 : (hc + 1) * 512], pk[:])
                    sqk = pA.tile([128, 1024], BF16, name="sqk", bufs=1)
                    nc.scalar.activation(sqk[:], kz[:], AF.Square)

                    def emit_pn(head):
                        pn = psA.tile([1, 512], F32, name="pn", bufs=2)
                        for c in range(2):
                            if head < 2:
                                rhs = sqq[:, (head * 2 + c) * 512 : (head * 2 + c + 1) * 512]
                                lhsT = invq[:, c : c + 1]
                            else:
                                rhs = sqk[:, c * 512 : (c + 1) * 512]
                                lhsT = invk[:, c : c + 1]
                            nc.tensor.matmul(
                                pn[:], lhsT, rhs, start=(c == 0), stop=(c == 1)
                            )
                        lnm = pA.tile([1, 512], F32, name="lnm", bufs=2)
                        nc.scalar.activation(
                            lnm[:], pn[:], AF.Ln, bias=epsb[:], scale=1.0 / 256.0
                        )
                        rst = pA.tile([1, 512], BF16, name="rst", bufs=2)
                        nc.scalar.activation(rst[:], lnm[:], AF.Exp, scale=-0.5)
                        return rst

                    rsts = [emit_pn(0), emit_pn(1)]
                    for sm in range(4):
                        pv = psA.tile([128, 256], F32, name="pvacc", bufs=2)
                        for kc in range(NKC):
                            nc.tensor.matmul(
                                pv[:],
                                hs_ap(g, kc, sm * 128, 128),
                                wv_sb[:, kc * 256 : (kc + 1) * 256],
                                start=(kc == 0),
                                stop=(kc == NKC - 1),
                            )
                        nc.scalar.copy(
                            v_s[g][:, sm * 256 : sm * 256 + 256], pv[:]
                        )

                    rsts.append(emit_pn(2))

                    # tail (rstd broadcast + rope) for the PREVIOUS s-tile:
                    # its norm chain has finished, so the pbc matmuls never
                    # stall PE, and rope (DVE) runs under this tile's
                    # projections.
                    def make_tail(g, qz, kz, rsts):
                        s0 = g * 512

                        def tail():
                            bcs = []
                            for head in range(3):
                                pbc = psA.tile([128, 512], F32, name="pbc", bufs=2)
                                nc.tensor.matmul(
                                    pbc[:], oner[:], rsts[head][:], start=True, stop=True
                                )
                                bc = pA.tile([128, 512], BF16, name=f"bc{head}", bufs=1)
                                nc.vector.tensor_copy(bc[:], pbc[:])
                                bcs.append(bc)
                            cs = cos_sb[:, s0 : s0 + 512]
                            sn = sin_sb[:, s0 : s0 + 512]

                            def rope2(z0, z1, bc, d0, d1):
                                t0 = pA.tile([128, 512], BF16, name="t0", bufs=1)
                                nc.vector.tensor_mul(t0[:], z0, cs)
                                t1 = pA.tile([128, 512], BF16, name="t1", bufs=1)
                                nc.vector.tensor_mul(t1[:], z1, sn)
                                u0 = pA.tile([128, 512], BF16, name="u0", bufs=1)
                                nc.vector.tensor_sub(u0[:], t0[:], t1[:])
                                nc.vector.tensor_mul(d0, u0[:], bc[:])
                                t2 = pA.tile([128, 512], BF16, name="t2", bufs=1)
                                nc.vector.tensor_mul(t2[:], z1, cs)
                                t3 = pA.tile([128, 512], BF16, name="t3", bufs=1)
                                nc.vector.tensor_mul(t3[:], z0, sn)
                                u1 = pA.tile([128, 512], BF16, name="u1", bufs=1)
                                nc.vector.tensor_add(u1[:], t2[:], t3[:])
                                nc.vector.tensor_mul(d1, u1[:], bc[:])

                            for head in range(2):
                                rope2(
                                    qz[:, (head * 2) * 512 : (head * 2) * 512 + 512],
                                    qz[:, (head * 2 + 1) * 512 : (head * 2 + 1) * 512 + 512],
                                    bcs[head],
                                    qTs[g][:, (head * 2) * 512 : (head * 2) * 512 + 512],
                                    qTs[g][:, (head * 2 + 1) * 512 : (head * 2 + 1) * 512 + 512],
                                )
                            rope2(
                                kz[:, 0:512], kz[:, 512:1024], bcs[2],
                                kTrs[g][:, 0:512],
                                kTrs[g][:, 512:1024],
                            )
                            nc.vector.tensor_mul(
                                kTns[g][:, 0:512], kz[:, 0:512], bcs[2][:]
                            )
                            nc.vector.tensor_mul(
                                kTns[g][:, 512:1024], kz[:, 512:1024], bcs[2][:]
                            )

                        return tail

                    if prev_tail is not None:
                        prev_tail()
                    prev_tail = make_tail(g, qz, kz, rsts)
                prev_tail()

            # ================= PHASES C+D ================================
            # wo streams into the SBUF space the phase-A pool released.
            with tc.tile_pool(name="pWo", bufs=1) as pWo:
                wo_sb = pWo.tile([128, 16 * HID], BF16, name="wo_sb")
                for i in range(4):
                    nc.sync.dma_start(
                        wo_sb[:, i * 4 * HID : (i + 1) * 4 * HID],
                        wop[:, i * 4 * HID : (i + 1) * 4 * HID],
                    )

                # ================= PHASE C: attention ====================
                with tc.tile_pool(name="pCps", bufs=1, space="PSUM") as psC:
                    # Pre-create every psum name in an order that maps the
                    # first q-tile's tiles (pss A/B, psm, po0 A, po1 A) onto
                    # phase-A banks whose last readers finish mid-tile
                    # (pacc/pvacc/pn, read by ACT), while the banks held
                    # until the phase-A tail's DVE reads (pbc) are only
                    # reused from the second q-tile on (po0/po1 buffer B)
                    # or by pbcC (first used at the first normalize).
                    for _ in range(2):
                        psC.tile([128, 512], F32, name="pss", bufs=2)
                    psC.tile([1, 512], F32, name="psm", bufs=1)
                    psC.tile([128, 512], F32, name="po0", bufs=2)
                    psC.tile([128, 512], F32, name="po1", bufs=2)
                    psC.tile([128, 512], F32, name="po0", bufs=2)
                    psC.tile([128, 512], F32, name="po1", bufs=2)
                    psC.tile([128, 512], F32, name="pbcC", bufs=1)
                    prev_norm = None
                    for stream in range(2):  # 0 = vanilla (roped k), 1 = lyra
                        kTg = kTrs if stream == 0 else kTns
                        # Q0 is all causal-masked tiles; run it last so the
                        # exp->select chain is warm and pipelined when it hits.
                        for Q in (1, 2, 3, 0):
                            tiles = _c_tiles(Q)
                            po0 = psC.tile([128, 512], F32, name="po0", bufs=2)
                            po1 = psC.tile([128, 512], F32, name="po1", bufs=2)
                            psm = psC.tile([1, 512], F32, name="psm", bufs=1)
                            probs_t = {}

                            def emit_scores(i):
                                T, off, ln, sel = tiles[i]
                                kT = kTg[T // 4]
                                tl = (T % 4) * 128
                                pss = psC.tile([128, 512], F32, name="pss", bufs=2)
                                for c in range(2):
                                    qb = (2 * stream + c) * 512 + off
                                    nc.tensor.matmul(
                                        pss[:, off : off + ln],
                                        kT[:, c * 512 + tl : c * 512 + tl + 128],
                                        qTs[Q][:, qb : qb + ln],
                                        start=(c == 0),
                                        stop=(c == 1),
                                    )
                                probs = pC.tile([128, 512], BF16, name="probs", bufs=5)
                                if sel is None:
                                    nc.scalar.activation(
                                        probs[:, off : off + ln],
                                        pss[:, off : off + ln],
                                        AF.Exp,
                                        scale=SCALING,
                                    )
                                else:
                                    pattern, base, cm = sel
                                    praw = pC.tile([128, 512], BF16, name="praw", bufs=2)
                                    nc.scalar.activation(
                                        praw[:, off : off + ln],
                                        pss[:, off : off + ln],
                                        AF.Exp,
                                        scale=SCALING,
                                    )
                                    nc.gpsimd.affine_select(
                                        probs[:, off : off + ln],
                                        praw[:, off : off + ln],
                                        pattern=pattern,
                                        compare_op=ALU.is_ge,
                                        fill=zero_fill,
                                        base=base,
                                        channel_multiplier=cm,
                                    )
                                probs_t[i] = probs

                            def emit_av(i):
                                T, off, ln, _ = tiles[i]
                                probs = probs_t.pop(i)
                                vt = v_s[T // 4]
                                tl = (T % 4) * 256
                                first = i == 0
                                last = i == len(tiles) - 1
                                nc.tensor.matmul(
                                    psm[:, off : off + ln],
                                    onec[:],
                                    probs[:, off : off + ln],
                                    start=first,
                                    stop=last,
                                    skip_group_check=True,
                                )
                                nc.tensor.matmul(
                                    po0[:, off : off + ln],
                                    vt[:, tl : tl + 128],
                                    probs[:, off : off + ln],
                                    start=first,
                                    stop=last,
                                    skip_group_check=True,
                                )
                                nc.tensor.matmul(
                                    po1[:, off : off + ln],
                                    vt[:, tl + 128 : tl + 256],
                                    probs[:, off : off + ln],
                                    start=first,
                                    stop=last,
                                    skip_group_check=True,
                                )

                            # software pipeline: AV for tile i trails the
                            # scores for tile i+2 so PE never waits on the
                            # ACT exp / GpSimd mask chain; the previous
                            # q-tile's normalize chain is emitted after the
                            # first scores block for the same reason.
                            for i in range(len(tiles)):
                                emit_scores(i)
                                if i == 0 and prev_norm is not None:
                                    prev_norm()
                                if i >= 3:
                                    emit_av(i - 3)
                            emit_av(len(tiles) - 3)
                            emit_av(len(tiles) - 2)
                            emit_av(len(tiles) - 1)
                            # free the single psm bank ASAP (ACT copy); the
                            # rest of the normalize chain is deferred into
                            # the next q-tile's score stream.
                            psmb = pC.tile([1, 512], BF16, name="psmb", bufs=2)
                            nc.scalar.copy(psmb[:], psm[:])

                            def make_norm(stream, Q, po0, po1, psmb):
                                def norm():
                                    pbcC = psC.tile(
                                        [128, 512], F32, name="pbcC", bufs=1
                                    )
                                    nc.tensor.matmul(
                                        pbcC[:], oner[:], psmb[:], start=True, stop=True
                                    )
                                    # 1/x as exp(-ln(x)), both on ACT
                                    lnC = pC.tile([128, 512], F32, name="lnC", bufs=2)
                                    nc.scalar.activation(lnC[:], pbcC[:], AF.Ln)
                                    bcsC = pC.tile([128, 512], F32, name="bcsC", bufs=2)
                                    nc.scalar.activation(
                                        bcsC[:], lnC[:], AF.Exp, scale=-1.0
                                    )
                                    for dc in range(2):
                                        po = po0 if dc == 0 else po1
                                        nc.vector.tensor_mul(
                                            outC[stream][dc][:, Q * 512 : (Q + 1) * 512],
                                            po[:],
                                            bcsC[:],
                                        )

                                return norm

                            prev_norm = make_norm(stream, Q, po0, po1, psmb)
                    prev_norm()

                # ================= PHASE D: output projection ============
                with tc.tile_pool(name="pDps", bufs=1, space="PSUM") as psD:
                    # outC is stored contiguously in query order; the lhsT
                    # for contraction chunk (j, dc) is the stride-8 view
                    # q = 8m + j over rows m0..m0+128. m-block-major with
                    # copies deferred one block so only the last block's
                    # copy+DMA is exposed.
                    prev_copy = None
                    for co in range(0, HID, 512):
                        for m in range(4):
                            stream, m0 = m // 2, (m % 2) * 128
                            pos = psD.tile([128, 512], F32, name=f"pD{m}", bufs=2)
                            final = co == HID - 512 and m == 3
                            if final:
                                # last block: two half-width accumulations so
                                # the first half's copy+DMA hides under the
                                # second half's matmuls, shrinking the drain.
                                if prev_copy is not None:
                                    prev_copy()
                                    prev_copy = None
                                for hh in range(2):
                                    for kc in range(16):
                                        j, dc = kc // 2, kc % 2
                                        lhsT = (
                                            outC[stream][dc][:]
                                            .rearrange("p (m j) -> p m j", j=8)
                                            [:, m0 : m0 + 128, j : j + 1]
                                        )
                                        nc.tensor.matmul(
                                            pos[:, hh * 256 : (hh + 1) * 256],
                                            lhsT,
                                            wo_sb[
                                                :,
                                                kc * HID + co + hh * 256 : kc * HID
                                                + co
                                                + (hh + 1) * 256,
                                            ],
                                            start=(kc == 0),
                                            stop=(kc == 15),
                                            skip_group_check=True,
                                        )
                                    osth = pC.tile(
                                        [128, 256], F32, name="osth", bufs=2
                                    )
                                    nc.scalar.copy(
                                        osth[:], pos[:, hh * 256 : (hh + 1) * 256]
                                    )
                                    nc.sync.dma_start(
                                        out_d[
                                            m * 128 : (m + 1) * 128,
                                            co + hh * 256 : co + (hh + 1) * 256,
                                        ],
                                        osth[:],
                                    )
                                continue
                            for kc in range(16):
                                j, dc = kc // 2, kc % 2
                                lhsT = (
                                    outC[stream][dc][:]
                                    .rearrange("p (m j) -> p m j", j=8)
                                    [:, m0 : m0 + 128, j : j + 1]
                                )
                                nc.tensor.matmul(
                                    pos[:],
                                    lhsT,
                                    wo_sb[:, kc * HID + co : kc * HID + co + 512],
                                    start=(kc == 0),
                                    stop=(kc == 15),
                                )

                            def make_copy(co, m, pos):
                                def docopy():
                                    ost = pC.tile(
                                        [128, 512], F32, name="ost", bufs=5
                                    )
                                    nc.scalar.copy(ost[:], pos[:])
                                    nc.sync.dma_start(
                                        out_d[
                                            m * 128 : (m + 1) * 128, co : co + 512
                                        ],
                                        ost[:],
                                    )

                                return docopy

                            if prev_copy is not None:
                                prev_copy()
                            prev_copy = make_copy(co, m, pos)
                    if prev_copy is not None:
                        prev_copy()
            pCt.__exit__(None, None, None)
    return nc


def _host_inputs(hidden_states, wq, wk, wv, wo, q_norm_w, k_norm_w):
    """Build the 8 per-core input maps (all host-side numpy prep).
    Every tensor is prepacked into its exact SBUF layout so device DMAs
    are plain contiguous copies."""
    hs = np.asarray(hidden_states, dtype=np.float32)
    wq = np.asarray(wq, dtype=np.float32)
    wk = np.asarray(wk, dtype=np.float32)
    wv = np.asarray(wv, dtype=np.float32)
    wo = np.asarray(wo, dtype=np.float32)
    qnw = np.asarray(q_norm_w, dtype=np.float32)
    knw = np.asarray(k_norm_w, dtype=np.float32)

    def pack_w(w):  # [HID, width] -> [128, NKC*width] chunk-major free axis
        width = w.shape[1]
        return np.ascontiguousarray(
            w.reshape(NKC, 128, width).transpose(1, 0, 2).reshape(128, NKC * width)
        ).astype(NPBF)

    # hsT packed per s-tile: [128, (st, kc, 256)]
    hsp = []
    for b in range(B):
        h = hs[b].T.reshape(NKC, 128, 4, 512).transpose(1, 2, 0, 3)
        hsp.append(np.ascontiguousarray(h.reshape(128, 4 * NKC * 512)).astype(NPBF))

    inv_freq = 1.0 / (THETA ** (np.arange(0, D, 2, dtype=np.float32) / D))
    ang = np.outer(inv_freq, np.arange(S, dtype=np.float32))  # (128, S)
    cosp = np.ascontiguousarray(np.cos(ang)).astype(NPBF)
    sinp = np.ascontiguousarray(np.sin(ang)).astype(NPBF)

    invq = np.ascontiguousarray(((1.0 + qnw) ** -2).reshape(2, 128).T).astype(NPBF)
    invk = np.ascontiguousarray(((1.0 + knw) ** -2).reshape(2, 128).T).astype(NPBF)
    onec = np.ones((128, 1), NPBF)
    oner = np.ones((1, 128), NPBF)
    epsb = np.full((1, 1), EPS, np.float32)

    # wo packed: [128, (kc, HID)]
    wop = np.ascontiguousarray(
        wo.reshape(16, 128, HID).transpose(1, 0, 2).reshape(128, 16 * HID)
    ).astype(NPBF)

    qs = 1.0 + qnw
    ks = 1.0 + knw
    in_maps = []
    for core in range(8):
        b, h = core // 4, core % 4
        wq2 = np.concatenate(
            [
                wq[:, h * D : (h + 1) * D] * qs[None, :],
                wq[:, (4 + h) * D : (5 + h) * D] * qs[None, :],
            ],
            axis=1,
        )  # [HID, 512]
        wqp_ = np.stack([pack_w(wq2[:, hc * 128 : (hc + 1) * 128]) for hc in range(4)])
        wk1 = wk[:, h * D : (h + 1) * D] * ks[None, :]
        wkp_ = np.stack([pack_w(wk1[:, hc * 128 : (hc + 1) * 128]) for hc in range(2)])
        wvp_ = pack_w(wv[:, h * D : (h + 1) * D])
        in_maps.append(
            {
                "hsp": hsp[b],
                "wqp": wqp_,
                "wkp": wkp_,
                "wvp": wvp_,
                "wop": wop,
                "cosp": cosp,
                "sinp": sinp,
                "invq": invq,
                "invk": invk,
                "onec": onec,
                "oner": oner,
                "epsb": epsb,
            }
        )
    return in_maps


_PROGRAM = None


def kernel(hidden_states, wq, wk, wv, wo, q_norm_w, k_norm_w):
    global _PROGRAM
    from concourse.bass_utils import run_bass_kernel_spmd

    if _PROGRAM is None:
        _PROGRAM = build_program()
    in_maps = _host_inputs(hidden_states, wq, wk, wv, wo, q_norm_w, k_norm_w)
    res = run_bass_kernel_spmd(_PROGRAM, in_maps, core_ids=list(range(8)))
    out = np.empty((B, S, HID), np.float32)
    for core in range(8):
        b, h = core // 4, core % 4
        out[b, h * 512 : (h + 1) * 512, :] = res.results[core]["out"]
    return out
